# revision 37
# baseline (speedup 1.0000x reference)
"""BoxAttention TRN2 kernel — 8-core data-parallel over the window dim.

Per core: 256 windows x 64 tokens x 384 dim, 12 heads, head_dim 32.
Pipeline per 128-token pair-tile (2 windows), all layouts chosen so no
gather and no cross-core communication is needed:

  xT  (c,tok)  <- DMA-transpose (bf16) or PE-transpose (f32) of x
  qT,kT (kout,tok) <- W_qk^T stationary matmuls, rhs = xT
  v   (tok,kv) <- xT-slice stationary matmuls, rhs = W_v^T  (+ ones col)
  S^T (nk,nq)  <- per (window,head) matmuls, tile_position packed
  E^T          <- exp(S^T) * exp(bias)^T            (bias folded via exp)
  AV  (nq, h*33+d) <- stationary E^T, moving v_ext; col 32 = softmax denom
  attn (tok,c) <- AV * 1/denom
  out (tok,o)  <- attn^T stationary, rhs = W_p^T, + proj_b
"""

import os
import sys
import numpy as np

for _p in ("/opt/trn_rl_repo", "/opt/pypackages"):
    if _p not in sys.path and os.path.isdir(_p):
        sys.path.append(_p)

import ml_dtypes  # noqa: E402

DIM, BOX, H = 384, 4, 12
N = BOX ** 3            # 64 tokens per window
HD = DIM // H           # 32
SCALE = HD ** -0.5
B_ = 2048
NCORES = 8
B_PER = B_ // NCORES    # 256 windows per core
TOK = B_PER * N         # 16384 tokens per core
SUPER = 512             # tokens per super-tile (8 windows)
PAIR = 128              # tokens per pair-tile (2 windows)

MODE = os.environ.get("BOXATTN_MODE", "f32")  # "f32" | "bf16" | "v2" | "v2pe"

_cache = {}


def _build_v2(tok_per_core, reps=1, avt_pe=False, dma_split=False, v4=False,
              bank_major=False, pool_mul=False, v6=False, pipe_tail=False):
    """Window-split bf16 kernel: S/AV computed per (window, head) with
    tile_position quadrant packing — no cross-window garbage blocks, so
    exp/bias-mul/AV contraction are half the work of the pair-packed
    layout. exp is batched over 2 pair-tiles (one [128,384] ACT op per
    PSUM row-group bank).

    Layouts per 2-pair group (pairs pr=0,1; windows w=0,1; head h=(g,j)
    with g=h%4, j=h//4):
      stg[g][64w:64w+64, pr*192+j*64+q] = S^T[k, q] of (pair pr, w, h)
      et2  [128, (g, pr*192+j*64+q)]    = exp(S^T) * exp(bias)
      avp  [tok128, h, 0:33]            = unnormalized AV | denominator
    """
    import concourse.bass as bass
    import concourse.mybir as mybir
    import concourse.tile as tile
    from concourse import bacc

    f32 = mybir.dt.float32
    dt = mybir.dt.bfloat16

    nc = bacc.Bacc("TRN2", target_bir_lowering=False, debug=False)

    x_d = nc.dram_tensor("x", [tok_per_core, DIM], dt, kind="ExternalInput").ap()
    wqk_d = nc.dram_tensor("wqkT", [DIM, 768], dt, kind="ExternalInput").ap()
    wv_d = nc.dram_tensor("wvT", [DIM, DIM], dt, kind="ExternalInput").ap()
    wp_d = nc.dram_tensor("wpT", [DIM, DIM], dt, kind="ExternalInput").ap()
    eb_d = nc.dram_tensor("ebT", [128, 1536], dt, kind="ExternalInput").ap()
    pb_d = nc.dram_tensor("pb", [PAIR, DIM], f32, kind="ExternalInput").ap()
    id_d = nc.dram_tensor("ident", [PAIR, PAIR], dt, kind="ExternalInput").ap()
    out_d = nc.dram_tensor("out", [tok_per_core, DIM], f32, kind="ExternalOutput").ap()

    n_super = tok_per_core // SUPER

    with tile.TileContext(nc) as tc:
        with (
            tc.tile_pool(name="consts", bufs=1) as consts,
            tc.tile_pool(name="xt", bufs=4 if v4 else 3) as xt_pool,
            tc.tile_pool(name="qk", bufs=3) as qk_pool,
            tc.tile_pool(name="er", bufs=3) as er_pool,
            tc.tile_pool(name="et", bufs=3) as et_pool,
            tc.tile_pool(name="av", bufs=4) as av_pool,
            tc.tile_pool(name="avt", bufs=4) as avt_pool,
            tc.tile_pool(name="osb", bufs=4) as o_pool,
            tc.tile_pool(name="inv", bufs=4) as inv_pool,
            tc.tile_pool(name="psA", bufs=2, space="PSUM") as psA,
            tc.tile_pool(name="psS", bufs=4, space="PSUM") as psS,
            tc.tile_pool(name="psV", bufs=2, space="PSUM") as psV,
        ):
            wqk = consts.tile([128, 3, 768], dt)
            nc.sync.dma_start(wqk[:], wqk_d.rearrange("(a p) k -> p a k", p=128))
            wv = consts.tile([128, 3, DIM], dt)
            nc.sync.dma_start(wv[:], wv_d.rearrange("(a p) k -> p a k", p=128))
            wp = consts.tile([128, 3, DIM], dt)
            nc.sync.dma_start(wp[:], wp_d.rearrange("(a p) k -> p a k", p=128))
            eb2 = consts.tile([128, 4, 384], dt)
            nc.sync.dma_start(eb2[:], eb_d.rearrange("p (a k) -> p a k", a=4))
            pb = consts.tile([PAIR, DIM], f32)
            nc.sync.dma_start(pb[:], pb_d)
            ident = None
            if avt_pe:
                ident = consts.tile([PAIR, PAIR], dt)
                nc.sync.dma_start(ident[:], id_d)
            nvbuf = 6 if v4 else 3
            vbufs = []
            for _i in range(nvbuf):
                vper = consts.tile([128, H, 33], dt, tag=f"vper{_i}")
                nc.vector.memset(vper[:, :, 32:33], 1.0)
                vbufs.append(vper)

            for sp in range(n_super * reps):
                t0 = (sp % n_super) * SUPER
                xt = xt_pool.tile([128, 3, SUPER], dt, tag="xt")
                for cc in range(3):
                    nc.sync.dma_start(
                        out=xt[:, cc, :],
                        in_=x_d[t0 : t0 + SUPER, cc * 128 : (cc + 1) * 128],
                        transpose=True,
                    )

                qkt = qk_pool.tile([128, 6, SUPER], dt, tag="qkt")
                for j in range(6):
                    ps = psA.tile([128, SUPER], f32, tag="psA")
                    for cc in range(3):
                        nc.tensor.matmul(
                            ps[:],
                            lhsT=wqk[:, cc, j * 128 : (j + 1) * 128],
                            rhs=xt[:, cc, :],
                            start=(cc == 0),
                            stop=(cc == 2),
                        )
                    nc.scalar.copy(qkt[:, j, :], ps[:])

                if v4:
                    # hoist all v projections: releases xt for next-super
                    # prefetch half a super earlier
                    for blk in range(4):
                        vps = psA.tile([128, DIM], f32, tag="psA")
                        for cc in range(3):
                            nc.tensor.matmul(
                                vps[:],
                                lhsT=xt[:, cc, blk * 128 : (blk + 1) * 128],
                                rhs=wv[:, cc, :],
                                start=(cc == 0),
                                stop=(cc == 2),
                            )
                        vsb = vbufs[(sp * 4 + blk) % nvbuf]
                        nc.vector.tensor_copy(
                            vsb[:, :, 0:32], vps[:].rearrange("p (h d) -> p h d", d=32)
                        )
                    osb_sup = o_pool.tile([128, 4, DIM], dt if v6 else f32, tag="osb")

                for half in range(2):
                    stg = []
                    for _g in range(4):
                        st_g = psS.tile([128, 384], f32, tag="s")
                        stg.append(st_g)
                    if bank_major:
                        # Emit S matmuls bank-major and fire exp(g)+mul(g) as
                        # soon as bank g completes, so ACT/DVE overlap the
                        # remaining banks' matmuls.
                        er2 = er_pool.tile([128, 4, 384], dt, tag="er")
                        et2 = et_pool.tile([128, 4, 384], dt, tag="et")
                        for g in range(4):
                            rp = g * 32
                            for pr in range(2):
                                blk = 2 * half + pr
                                for j in range(3):
                                    h = 4 * j + g
                                    for w in range(2):
                                        f0 = blk * PAIR + w * N
                                        c0 = pr * 192 + j * 64
                                        nc.tensor.matmul(
                                            stg[g][w * N : (w + 1) * N, c0 : c0 + 64],
                                            lhsT=qkt[rp : rp + 32, 3 + j, f0 : f0 + N],
                                            rhs=qkt[rp : rp + 32, j, f0 : f0 + N],
                                            start=True,
                                            stop=True,
                                            tile_position=(rp, w * N),
                                        )
                            nc.scalar.activation(
                                er2[:, g, :], stg[g][:],
                                mybir.ActivationFunctionType.Exp,
                            )
                            mul_eng = nc.gpsimd if pool_mul else nc.vector
                            mul_eng.tensor_mul(
                                et2[:, g, :], er2[:, g, :], eb2[:, g, :]
                            )
                    for pr in range(2):
                        blk = 2 * half + pr
                        if not v4:
                            # ---- v (natural layout, ones col for denom) ----
                            vps = psA.tile([128, DIM], f32, tag="psA")
                            for cc in range(3):
                                nc.tensor.matmul(
                                    vps[:],
                                    lhsT=xt[:, cc, blk * 128 : (blk + 1) * 128],
                                    rhs=wv[:, cc, :],
                                    start=(cc == 0),
                                    stop=(cc == 2),
                                )
                            vsb = vbufs[(sp * 4 + blk) % nvbuf]
                            nc.vector.tensor_copy(
                                vsb[:, :, 0:32],
                                vps[:].rearrange("p (h d) -> p h d", d=32),
                            )
                        # ---- S^T per (window, head): [32,64]x[32,64] in a
                        # quadrant; w0 -> rows/psum-partitions 0:64, w1 ->
                        # 64:128. Different windows write disjoint partition
                        # halves of the same bank, so they can overlap.
                        if not bank_major:
                            for h in range(H):
                                g, j = h % 4, h // 4
                                rp = g * 32
                                for w in range(2):
                                    f0 = blk * PAIR + w * N
                                    c0 = pr * 192 + j * 64
                                    nc.tensor.matmul(
                                        stg[g][w * N : (w + 1) * N, c0 : c0 + 64],
                                        lhsT=qkt[rp : rp + 32, 3 + j, f0 : f0 + N],
                                        rhs=qkt[rp : rp + 32, j, f0 : f0 + N],
                                        start=True,
                                        stop=True,
                                        tile_position=(rp, w * N),
                                    )
                    if not bank_major:
                        # ---- exp over both pairs, one ACT op per bank ----
                        er2 = er_pool.tile([128, 4, 384], dt, tag="er")
                        for g in range(4):
                            nc.scalar.activation(
                                er2[:, g, :], stg[g][:],
                                mybir.ActivationFunctionType.Exp,
                            )
                        et2 = et_pool.tile([128, 4, 384], dt, tag="et")
                        nc.vector.tensor_mul(et2[:], er2[:], eb2[:])

                    for pr in range(2):
                        blk = 2 * half + pr
                        vsb = vbufs[(sp * 4 + blk) % nvbuf]
                        avp_t = psV.tile([128, H * 33], f32, tag="avp")
                        avp = avp_t[:].rearrange("p (h d) -> p h d", d=33)
                        for h in range(H):
                            g, j = h % 4, h // 4
                            c0 = pr * 192 + j * 64
                            for w in range(2):
                                nc.tensor.matmul(
                                    avp[w * N : (w + 1) * N, h, :],
                                    lhsT=et2[w * N : (w + 1) * N, g, c0 : c0 + 64],
                                    rhs=vsb[w * N : (w + 1) * N, h, 0:33],
                                    start=True,
                                    stop=True,
                                    tile_position=(w * N, w * N),
                                )
                        inv = inv_pool.tile([128, H], f32, tag="inv")
                        nc.vector.reciprocal(inv[:], avp[:, :, 32])
                        avsb = av_pool.tile([128, H, 32], dt, tag="av")
                        nc.vector.tensor_mul(
                            avsb[:],
                            avp[:, :, 0:32],
                            inv[:, :, None].broadcast_to([128, H, 32]),
                        )

                        avt = avt_pool.tile([128, 3, 128], dt, tag="avt")
                        late_dma = nc.scalar if dma_split else nc.sync
                        if avt_pe:
                            for cc in range(3):
                                tp = psV.tile([128, 128], dt, tag="avp")
                                nc.tensor.transpose(
                                    tp[:],
                                    avsb[:].rearrange("p h d -> p (h d)")[
                                        :, cc * 128 : (cc + 1) * 128
                                    ],
                                    ident[:],
                                )
                                nc.scalar.copy(avt[:, cc, :], tp[:])
                        else:
                            late_dma.dma_start(
                                out=avt[:],
                                in_=avsb[:].rearrange("p h d -> p (h d)"),
                                transpose=True,
                            )

                        if pipe_tail:
                            pending.append((avt, blk, osb_sup, t0))
                            while len(pending) > 2:
                                flush_pair(pending.pop(0))
                            continue
                        ops = psA.tile([128, DIM], f32, tag="psA")
                        for cc in range(3):
                            nc.tensor.matmul(
                                ops[:],
                                lhsT=avt[:, cc, :],
                                rhs=wp[:, cc, :],
                                start=(cc == 0),
                                stop=(cc == 2),
                            )
                        if v4:
                            if v6:
                                # proj bias is folded into wpT on host
                                # (softmax rows sum to 1 -> wp' = wp + pb/H)
                                nc.vector.tensor_copy(osb_sup[:, blk, :], ops[:])
                            else:
                                nc.vector.tensor_add(osb_sup[:, blk, :], ops[:], pb[:])
                        else:
                            osb = o_pool.tile([128, DIM], f32, tag="osb")
                            nc.vector.tensor_add(osb[:], ops[:], pb[:])
                            tok0 = t0 + blk * PAIR
                            late_dma.dma_start(out_d[tok0 : tok0 + PAIR, :], osb[:])
                if v4 and not pipe_tail:
                    out_ap = out_d[t0 : t0 + SUPER, :].rearrange(
                        "(b p) c -> p b c", p=128
                    )
                    if v6:
                        # SWDGE casts bf16 -> f32 during the store
                        nc.gpsimd.dma_start(out_ap, osb_sup[:])
                    else:
                        nc.sync.dma_start(out_ap, osb_sup[:])
            if pipe_tail:
                while pending:
                    flush_pair(pending.pop(0))
    nc.compile()
    return nc


def _build(mode, tok_per_core, reps=1):
    import concourse.bass as bass
    import concourse.mybir as mybir
    import concourse.tile as tile
    from concourse import bacc

    f32 = mybir.dt.float32
    dt = mybir.dt.bfloat16 if mode == "bf16" else f32

    nc = bacc.Bacc("TRN2", target_bir_lowering=False, debug=False)

    x_d = nc.dram_tensor("x", [tok_per_core, DIM], dt, kind="ExternalInput").ap()
    wqk_d = nc.dram_tensor("wqkT", [DIM, 768], dt, kind="ExternalInput").ap()
    wv_d = nc.dram_tensor("wvT", [DIM, DIM], dt, kind="ExternalInput").ap()
    wp_d = nc.dram_tensor("wpT", [DIM, DIM], dt, kind="ExternalInput").ap()
    eb_d = nc.dram_tensor("ebT", [PAIR, 1536], dt, kind="ExternalInput").ap()
    pb_d = nc.dram_tensor("pb", [PAIR, DIM], f32, kind="ExternalInput").ap()
    id_d = nc.dram_tensor("ident", [PAIR, PAIR], f32, kind="ExternalInput").ap()
    out_d = nc.dram_tensor("out", [tok_per_core, DIM], f32, kind="ExternalOutput").ap()

    n_super = tok_per_core // SUPER

    with tile.TileContext(nc) as tc:
        with (
            tc.tile_pool(name="consts", bufs=1) as consts,
            tc.tile_pool(name="xn", bufs=3) as xn_pool,
            tc.tile_pool(name="xt", bufs=3) as xt_pool,
            tc.tile_pool(name="qk", bufs=3) as qk_pool,
            tc.tile_pool(name="v", bufs=3) as v_pool,
            tc.tile_pool(name="er", bufs=4) as er_pool,
            tc.tile_pool(name="et", bufs=4) as et_pool,
            tc.tile_pool(name="av", bufs=4) as av_pool,
            tc.tile_pool(name="avt", bufs=4) as avt_pool,
            tc.tile_pool(name="osb", bufs=4) as o_pool,
            tc.tile_pool(name="inv", bufs=4) as inv_pool,
            tc.tile_pool(name="psA", bufs=2, space="PSUM") as psA,
            tc.tile_pool(name="psS", bufs=4, space="PSUM") as psS,
            tc.tile_pool(name="psB", bufs=2, space="PSUM") as psB,
        ):
            wqk = consts.tile([128, 3, 768], dt)
            nc.sync.dma_start(wqk[:], wqk_d.rearrange("(a p) k -> p a k", p=128))
            wv = consts.tile([128, 3, DIM], dt)
            nc.sync.dma_start(wv[:], wv_d.rearrange("(a p) k -> p a k", p=128))
            wp = consts.tile([128, 3, DIM], dt)
            nc.sync.dma_start(wp[:], wp_d.rearrange("(a p) k -> p a k", p=128))
            eb = consts.tile([PAIR, 1536], dt)
            nc.sync.dma_start(eb[:], eb_d)
            pb = consts.tile([PAIR, DIM], f32)
            nc.sync.dma_start(pb[:], pb_d)
            ident = None
            if mode != "bf16":
                ident = consts.tile([PAIR, PAIR], f32)
                nc.sync.dma_start(ident[:], id_d)
            vbufs = []
            for _i in range(3):
                vper = consts.tile([128, H, 33], dt, tag=f"vper{_i}")
                nc.vector.memset(vper[:, :, 32:33], 1.0)
                vbufs.append(vper)

            for sp in range(n_super * reps):
                t0 = (sp % n_super) * SUPER
                # ---- xT [c, tok] for this super-tile ----
                xt = xt_pool.tile([128, 3, SUPER], dt, tag="xt")
                if mode == "bf16":
                    for cc in range(3):
                        nc.sync.dma_start(
                            out=xt[:, cc, :],
                            in_=x_d[t0 : t0 + SUPER, cc * 128 : (cc + 1) * 128],
                            transpose=True,
                        )
                else:
                    xn = xn_pool.tile([128, 4, DIM], f32, tag="xn")
                    nc.sync.dma_start(
                        xn[:], x_d[t0 : t0 + SUPER, :].rearrange("(b p) c -> p b c", p=128)
                    )
                    for cc in range(3):
                        for tb in range(4):
                            tp = psB.tile([128, 128], f32, tag="bp")
                            nc.tensor.transpose(
                                tp[:], xn[:, tb, cc * 128 : (cc + 1) * 128], ident[:]
                            )
                            nc.scalar.copy(xt[:, cc, tb * 128 : (tb + 1) * 128], tp[:])

                # ---- q,k projections (transposed layout) ----
                qkt = qk_pool.tile([128, 6, SUPER], dt, tag="qkt")
                for j in range(6):
                    ps = psA.tile([128, SUPER], f32, tag="psA")
                    for cc in range(3):
                        nc.tensor.matmul(
                            ps[:],
                            lhsT=wqk[:, cc, j * 128 : (j + 1) * 128],
                            rhs=xt[:, cc, :],
                            start=(cc == 0),
                            stop=(cc == 2),
                        )
                    nc.scalar.copy(qkt[:, j, :], ps[:])

                for blk in range(4):
                    tok0 = t0 + blk * PAIR
                    # ---- v (natural layout, interleaved with ones col) ----
                    vps = psA.tile([128, DIM], f32, tag="psA")
                    for cc in range(3):
                        nc.tensor.matmul(
                            vps[:],
                            lhsT=xt[:, cc, blk * 128 : (blk + 1) * 128],
                            rhs=wv[:, cc, :],
                            start=(cc == 0),
                            stop=(cc == 2),
                        )
                    vsb = vbufs[(sp * 4 + blk) % 3]
                    nc.vector.tensor_copy(
                        vsb[:, :, 0:32], vps[:].rearrange("p (h d) -> p h d", d=32)
                    )

                    # ---- S^T per head: one [32,128]x[32,128] matmul over the
                    # whole pair-tile. Cross-window blocks are garbage; the
                    # bias multiply (eb = 0 there) zeroes them, which makes
                    # E^T block-diagonal so AV is one matmul per head too.
                    # One PSUM bank per PE row-group g=h%4 (concurrent
                    # tile_position matmuls must not share a bank).
                    # Bank g must hold exactly the heads of PE row-group g:
                    # concurrent tile_position matmuls from different row
                    # groups must not write the same PSUM bank.
                    sts = []
                    for _g in range(4):
                        st_g = psS.tile([128, 384], f32, tag="s")
                        sts.append(st_g)
                    for h in range(H):
                        g, j = h % 4, h // 4
                        rp = g * 32
                        f0 = blk * 128
                        nc.tensor.matmul(
                            sts[g][:, j * 128 : (j + 1) * 128],
                            lhsT=qkt[rp : rp + 32, 3 + j, f0 : f0 + 128],
                            rhs=qkt[rp : rp + 32, j, f0 : f0 + 128],
                            start=True,
                            stop=True,
                            tile_position=(rp, 0),
                        )
                    er = er_pool.tile([128, 1536], dt, tag="er")
                    for g in range(4):
                        nc.scalar.activation(
                            er[:, g * 384 : (g + 1) * 384],
                            sts[g][:],
                            mybir.ActivationFunctionType.Exp,
                        )
                    et = et_pool.tile([128, 1536], dt, tag="et")
                    nc.vector.tensor_mul(et[:], er[:], eb[:])

                    # ---- AV (+ denominator in col 32 of each head block) ----
                    avp_t = psB.tile([128, 512], f32, tag="bp")
                    avp = avp_t[:, 0 : H * 33].rearrange("p (h d) -> p h d", d=33)
                    for h in range(H):
                        ec = (h % 4) * 384 + (h // 4) * 128
                        nc.tensor.matmul(
                            avp[:, h, :],
                            lhsT=et[:, ec : ec + 128],
                            rhs=vsb[:, h, :],
                            start=True,
                            stop=True,
                        )
                    inv = inv_pool.tile([128, H], f32, tag="inv")
                    nc.vector.reciprocal(inv[:], avp[:, :, 32])
                    avsb = av_pool.tile([128, H, 32], dt, tag="av")
                    nc.vector.tensor_mul(
                        avsb[:],
                        avp[:, :, 0:32],
                        inv[:, :, None].broadcast_to([128, H, 32]),
                    )

                    # ---- attn^T for the output projection ----
                    avt = avt_pool.tile([128, 3, 128], dt, tag="avt")
                    if mode == "bf16":
                        nc.sync.dma_start(
                            out=avt[:],
                            in_=avsb[:].rearrange("p h d -> p (h d)"),
                            transpose=True,
                        )
                    else:
                        for cc in range(3):
                            tp = psB.tile([128, 128], f32, tag="bp")
                            nc.tensor.transpose(
                                tp[:],
                                avsb[:].rearrange("p h d -> p (h d)")[
                                    :, cc * 128 : (cc + 1) * 128
                                ],
                                ident[:],
                            )
                            nc.scalar.copy(avt[:, cc, :], tp[:])

                    # ---- output projection + bias ----
                    ops = psA.tile([128, DIM], f32, tag="psA")
                    for cc in range(3):
                        nc.tensor.matmul(
                            ops[:],
                            lhsT=avt[:, cc, :],
                            rhs=wp[:, cc, :],
                            start=(cc == 0),
                            stop=(cc == 2),
                        )
                    osb = o_pool.tile([128, DIM], f32, tag="osb")
                    nc.vector.tensor_add(osb[:], ops[:], pb[:])
                    nc.sync.dma_start(out_d[tok0 : tok0 + PAIR, :], osb[:])
    nc.compile()
    return nc


def _get_nc(mode, tok_per_core, reps=1):
    key = (mode, tok_per_core, reps)
    if key not in _cache:
        if mode.startswith(("v2", "v3", "v4", "v5", "v6")):
            _cache[key] = _build_v2(
                tok_per_core,
                reps,
                avt_pe=mode.endswith("pe"),
                dma_split=not mode.startswith("v2"),
                v4=mode.startswith(("v4", "v5", "v6")),
                bank_major=mode.startswith(("v5", "v6")),
                pool_mul=(mode == "v5p"),
                v6=mode.startswith("v6"),
            )
        else:
            _cache[key] = _build(mode, tok_per_core, reps)
    return _cache[key]


def _host_prep(x, qkv_w, proj_w, proj_b, bias_table, rel_idx, mode, n_cores):
    np_dt = np.float32 if mode == "f32" else ml_dtypes.bfloat16
    x = np.asarray(x, np.float32)
    qkv_w = np.asarray(qkv_w, np.float32)
    proj_w = np.asarray(proj_w, np.float32)
    proj_b = np.asarray(proj_b, np.float32)
    bias_table = np.asarray(bias_table, np.float32)
    rel_idx = np.asarray(rel_idx)

    wq = qkv_w[0:DIM] * SCALE
    wk = qkv_w[DIM : 2 * DIM]
    wv = qkv_w[2 * DIM :]
    wqkT = np.concatenate([wq, wk], 0).T.copy().astype(np_dt)  # [384, 768]
    wvT = wv.T.copy().astype(np_dt)
    wpT_f = proj_w.T.copy()
    if mode.startswith("v6"):
        # fold proj bias into the weights: each head's softmax row sums to 1,
        # so attn @ (wp + pb/H broadcast over c) = attn @ wp + pb
        wpT_f = wpT_f + proj_b[None, :] / H
    wpT = wpT_f.astype(np_dt)

    bias = bias_table[rel_idx.reshape(-1)].reshape(N, N, H)  # [nq, nk, h]
    eb1 = np.exp(bias).transpose(1, 2, 0)  # [nk, h, nq]
    if mode.startswith(("v2", "v3", "v4", "v5", "v6")):
        # eb2 [128, (g, pr, j, q)]: rows repeat at 64 (same for both windows)
        ebT = np.zeros((PAIR, 4, 2, 3, N), np.float32)
        for h in range(H):
            g, j = h % 4, h // 4
            for w in range(2):
                for pr in range(2):
                    ebT[w * N : (w + 1) * N, g, pr, j, :] = eb1[:, h, :]
        ebT = ebT.reshape(PAIR, H * PAIR).astype(np_dt)
        ident = np.eye(PAIR, dtype=np_dt)
    else:
        ebT = np.zeros((PAIR, H * PAIR), np.float32)  # cross-window blocks stay 0
        for h in range(H):
            ec = (h % 4) * 384 + (h // 4) * 128
            for w in range(2):
                ebT[w * N : (w + 1) * N, ec + w * N : ec + (w + 1) * N] = eb1[:, h, :]
        ebT = ebT.astype(np_dt)  # [128, 1536]
        ident = np.eye(PAIR, dtype=np.float32)
    pb = np.broadcast_to(proj_b, (PAIR, DIM)).copy().astype(np.float32)

    B = x.shape[0]
    bper = B // n_cores
    xs = x.reshape(B * N, DIM).astype(np_dt)
    in_maps = []
    for c in range(n_cores):
        in_maps.append(
            {
                "x": xs[c * bper * N : (c + 1) * bper * N],
                "wqkT": wqkT,
                "wvT": wvT,
                "wpT": wpT,
                "ebT": ebT,
                "pb": pb,
                "ident": ident,
            }
        )
    return in_maps


def kernel(x, qkv_w, proj_w, proj_b, bias_table, rel_idx):
    from concourse.bass_utils import run_bass_kernel_spmd

    x = np.asarray(x)
    B = x.shape[0]
    n_cores = NCORES
    tok_per_core = (B // n_cores) * N
    nc = _get_nc(MODE, tok_per_core)
    in_maps = _host_prep(x, qkv_w, proj_w, proj_b, bias_table, rel_idx, MODE, n_cores)
    res = run_bass_kernel_spmd(nc, in_maps, list(range(n_cores)))
    out = np.concatenate([r["out"] for r in res.results], 0)
    return out.reshape(B, N, DIM).astype(np.float32)



# revision 60
# speedup vs baseline: 1.2057x; 1.2057x over previous
"""BoxAttention TRN2 kernel — 8-core data-parallel over the window dim.

Per core: 256 windows x 64 tokens x 384 dim, 12 heads, head_dim 32.
Pipeline per 128-token pair-tile (2 windows), all layouts chosen so no
gather and no cross-core communication is needed:

  xT  (c,tok)  <- DMA-transpose (bf16) or PE-transpose (f32) of x
  qT,kT (kout,tok) <- W_qk^T stationary matmuls, rhs = xT
  v   (tok,kv) <- xT-slice stationary matmuls, rhs = W_v^T  (+ ones col)
  S^T (nk,nq)  <- per (window,head) matmuls, tile_position packed
  E^T          <- exp(S^T) * exp(bias)^T            (bias folded via exp)
  AV  (nq, h*33+d) <- stationary E^T, moving v_ext; col 32 = softmax denom
  attn (tok,c) <- AV * 1/denom
  out (tok,o)  <- attn^T stationary, rhs = W_p^T, + proj_b
"""

import os
import sys
import numpy as np

for _p in ("/opt/trn_rl_repo", "/opt/pypackages"):
    if _p not in sys.path and os.path.isdir(_p):
        sys.path.append(_p)

import ml_dtypes  # noqa: E402

DIM, BOX, H = 384, 4, 12
N = BOX ** 3            # 64 tokens per window
HD = DIM // H           # 32
SCALE = HD ** -0.5
B_ = 2048
NCORES = 8
B_PER = B_ // NCORES    # 256 windows per core
TOK = B_PER * N         # 16384 tokens per core
SUPER = 512             # tokens per super-tile (8 windows)
PAIR = 128              # tokens per pair-tile (2 windows)

MODE = os.environ.get("BOXATTN_MODE", "f32")  # "f32" | "bf16" | "v2" | "v2pe"

_cache = {}


def _build_v2(tok_per_core, reps=1, avt_pe=False, dma_split=False, v4=False,
              bank_major=False, pool_mul=False, v6=False, pipe_tail=False,
              cast_store=None):
    """Window-split bf16 kernel: S/AV computed per (window, head) with
    tile_position quadrant packing — no cross-window garbage blocks, so
    exp/bias-mul/AV contraction are half the work of the pair-packed
    layout. exp is batched over 2 pair-tiles (one [128,384] ACT op per
    PSUM row-group bank).

    Layouts per 2-pair group (pairs pr=0,1; windows w=0,1; head h=(g,j)
    with g=h%4, j=h//4):
      stg[g][64w:64w+64, pr*192+j*64+q] = S^T[k, q] of (pair pr, w, h)
      et2  [128, (g, pr*192+j*64+q)]    = exp(S^T) * exp(bias)
      avp  [tok128, h, 0:33]            = unnormalized AV | denominator
    """
    import concourse.bass as bass
    import concourse.mybir as mybir
    import concourse.tile as tile
    from concourse import bacc

    f32 = mybir.dt.float32
    dt = mybir.dt.bfloat16

    nc = bacc.Bacc("TRN2", target_bir_lowering=False, debug=False)

    x_d = nc.dram_tensor("x", [tok_per_core, DIM], dt, kind="ExternalInput").ap()
    wqk_d = nc.dram_tensor("wqkT", [DIM, 768], dt, kind="ExternalInput").ap()
    wv_d = nc.dram_tensor("wvT", [DIM, DIM], dt, kind="ExternalInput").ap()
    wp_d = nc.dram_tensor("wpT", [DIM, DIM], dt, kind="ExternalInput").ap()
    eb_d = nc.dram_tensor("ebT", [128, 1536], dt, kind="ExternalInput").ap()
    pb_d = nc.dram_tensor("pb", [PAIR, DIM], f32, kind="ExternalInput").ap()
    id_d = nc.dram_tensor("ident", [PAIR, PAIR], dt, kind="ExternalInput").ap()
    out_d = nc.dram_tensor("out", [tok_per_core, DIM], f32, kind="ExternalOutput").ap()

    if cast_store is None:
        cast_store = v6
    n_super = tok_per_core // SUPER

    with tile.TileContext(nc) as tc:
        with (
            tc.tile_pool(name="consts", bufs=1) as consts,
            tc.tile_pool(name="xt", bufs=4 if v4 else 3) as xt_pool,
            tc.tile_pool(name="qk", bufs=3) as qk_pool,
            tc.tile_pool(name="er", bufs=3) as er_pool,
            tc.tile_pool(name="et", bufs=3) as et_pool,
            tc.tile_pool(name="av", bufs=4) as av_pool,
            tc.tile_pool(name="avt", bufs=4) as avt_pool,
            tc.tile_pool(name="osb", bufs=4) as o_pool,
            tc.tile_pool(name="inv", bufs=4) as inv_pool,
            tc.tile_pool(name="psA", bufs=2, space="PSUM") as psA,
            tc.tile_pool(name="psS", bufs=4, space="PSUM") as psS,
            tc.tile_pool(name="psV", bufs=2, space="PSUM") as psV,
        ):
            wqk = consts.tile([128, 3, 768], dt)
            nc.sync.dma_start(wqk[:], wqk_d.rearrange("(a p) k -> p a k", p=128))
            wv = consts.tile([128, 3, DIM], dt)
            nc.sync.dma_start(wv[:], wv_d.rearrange("(a p) k -> p a k", p=128))
            wp = consts.tile([128, 3, DIM], dt)
            nc.sync.dma_start(wp[:], wp_d.rearrange("(a p) k -> p a k", p=128))
            eb2 = consts.tile([128, 4, 384], dt)
            nc.sync.dma_start(eb2[:], eb_d.rearrange("p (a k) -> p a k", a=4))
            pb = consts.tile([PAIR, DIM], f32)
            nc.sync.dma_start(pb[:], pb_d)

            pending = []

            def flush_pair(item):
                f_avt, f_blk, f_osb, f_t0 = item
                ops = psA.tile([128, DIM], f32, tag="psA")
                for cc in range(3):
                    nc.tensor.matmul(
                        ops[:],
                        lhsT=f_avt[:, cc, :],
                        rhs=wp[:, cc, :],
                        start=(cc == 0),
                        stop=(cc == 2),
                    )
                nc.vector.tensor_add(f_osb[:, f_blk, :], ops[:], pb[:])
                if f_blk == 3:
                    out_ap = out_d[f_t0 : f_t0 + SUPER, :].rearrange(
                        "(b p) c -> p b c", p=128
                    )
                    if cast_store:
                        nc.gpsimd.dma_start(out_ap, f_osb[:])
                    else:
                        nc.sync.dma_start(out_ap, f_osb[:])
            ident = None
            if avt_pe:
                ident = consts.tile([PAIR, PAIR], dt)
                nc.sync.dma_start(ident[:], id_d)
            nvbuf = 6 if v4 else 3
            vbufs = []
            for _i in range(nvbuf):
                vper = consts.tile([128, H, 33], dt, tag=f"vper{_i}")
                nc.vector.memset(vper[:, :, 32:33], 1.0)
                vbufs.append(vper)

            for sp in range(n_super * reps):
                t0 = (sp % n_super) * SUPER
                xt = xt_pool.tile([128, 3, SUPER], dt, tag="xt")
                for cc in range(3):
                    nc.sync.dma_start(
                        out=xt[:, cc, :],
                        in_=x_d[t0 : t0 + SUPER, cc * 128 : (cc + 1) * 128],
                        transpose=True,
                    )

                qkt = qk_pool.tile([128, 6, SUPER], dt, tag="qkt")
                for j in range(6):
                    ps = psA.tile([128, SUPER], f32, tag="psA")
                    for cc in range(3):
                        nc.tensor.matmul(
                            ps[:],
                            lhsT=wqk[:, cc, j * 128 : (j + 1) * 128],
                            rhs=xt[:, cc, :],
                            start=(cc == 0),
                            stop=(cc == 2),
                        )
                    nc.scalar.copy(qkt[:, j, :], ps[:])

                if v4:
                    # hoist all v projections: releases xt for next-super
                    # prefetch half a super earlier
                    for blk in range(4):
                        vps = psA.tile([128, DIM], f32, tag="psA")
                        for cc in range(3):
                            nc.tensor.matmul(
                                vps[:],
                                lhsT=xt[:, cc, blk * 128 : (blk + 1) * 128],
                                rhs=wv[:, cc, :],
                                start=(cc == 0),
                                stop=(cc == 2),
                            )
                        vsb = vbufs[(sp * 4 + blk) % nvbuf]
                        nc.vector.tensor_copy(
                            vsb[:, :, 0:32], vps[:].rearrange("p (h d) -> p h d", d=32)
                        )
                    osb_sup = o_pool.tile([128, 4, DIM], dt if cast_store else f32,
                                          tag="osb")

                for half in range(2):
                    stg = []
                    for _g in range(4):
                        st_g = psS.tile([128, 384], f32, tag="s")
                        stg.append(st_g)
                    if bank_major:
                        # Emit S matmuls bank-major and fire exp(g)+mul(g) as
                        # soon as bank g completes, so ACT/DVE overlap the
                        # remaining banks' matmuls.
                        er2 = er_pool.tile([128, 4, 384], dt, tag="er")
                        et2 = et_pool.tile([128, 4, 384], dt, tag="et")
                        for g in range(4):
                            rp = g * 32
                            for pr in range(2):
                                blk = 2 * half + pr
                                for j in range(3):
                                    h = 4 * j + g
                                    for w in range(2):
                                        f0 = blk * PAIR + w * N
                                        c0 = pr * 192 + j * 64
                                        nc.tensor.matmul(
                                            stg[g][w * N : (w + 1) * N, c0 : c0 + 64],
                                            lhsT=qkt[rp : rp + 32, 3 + j, f0 : f0 + N],
                                            rhs=qkt[rp : rp + 32, j, f0 : f0 + N],
                                            start=True,
                                            stop=True,
                                            tile_position=(rp, w * N),
                                        )
                            nc.scalar.activation(
                                er2[:, g, :], stg[g][:],
                                mybir.ActivationFunctionType.Exp,
                            )
                            mul_eng = nc.gpsimd if pool_mul else nc.vector
                            mul_eng.tensor_mul(
                                et2[:, g, :], er2[:, g, :], eb2[:, g, :]
                            )
                    for pr in range(2):
                        blk = 2 * half + pr
                        if not v4:
                            # ---- v (natural layout, ones col for denom) ----
                            vps = psA.tile([128, DIM], f32, tag="psA")
                            for cc in range(3):
                                nc.tensor.matmul(
                                    vps[:],
                                    lhsT=xt[:, cc, blk * 128 : (blk + 1) * 128],
                                    rhs=wv[:, cc, :],
                                    start=(cc == 0),
                                    stop=(cc == 2),
                                )
                            vsb = vbufs[(sp * 4 + blk) % nvbuf]
                            nc.vector.tensor_copy(
                                vsb[:, :, 0:32],
                                vps[:].rearrange("p (h d) -> p h d", d=32),
                            )
                        # ---- S^T per (window, head): [32,64]x[32,64] in a
                        # quadrant; w0 -> rows/psum-partitions 0:64, w1 ->
                        # 64:128. Different windows write disjoint partition
                        # halves of the same bank, so they can overlap.
                        if not bank_major:
                            for h in range(H):
                                g, j = h % 4, h // 4
                                rp = g * 32
                                for w in range(2):
                                    f0 = blk * PAIR + w * N
                                    c0 = pr * 192 + j * 64
                                    nc.tensor.matmul(
                                        stg[g][w * N : (w + 1) * N, c0 : c0 + 64],
                                        lhsT=qkt[rp : rp + 32, 3 + j, f0 : f0 + N],
                                        rhs=qkt[rp : rp + 32, j, f0 : f0 + N],
                                        start=True,
                                        stop=True,
                                        tile_position=(rp, w * N),
                                    )
                    if not bank_major:
                        # ---- exp over both pairs, one ACT op per bank ----
                        er2 = er_pool.tile([128, 4, 384], dt, tag="er")
                        for g in range(4):
                            nc.scalar.activation(
                                er2[:, g, :], stg[g][:],
                                mybir.ActivationFunctionType.Exp,
                            )
                        et2 = et_pool.tile([128, 4, 384], dt, tag="et")
                        nc.vector.tensor_mul(et2[:], er2[:], eb2[:])

                    for pr in range(2):
                        blk = 2 * half + pr
                        vsb = vbufs[(sp * 4 + blk) % nvbuf]
                        avp_t = psV.tile([128, H * 33], f32, tag="avp")
                        avp = avp_t[:].rearrange("p (h d) -> p h d", d=33)
                        for h in range(H):
                            g, j = h % 4, h // 4
                            c0 = pr * 192 + j * 64
                            for w in range(2):
                                nc.tensor.matmul(
                                    avp[w * N : (w + 1) * N, h, :],
                                    lhsT=et2[w * N : (w + 1) * N, g, c0 : c0 + 64],
                                    rhs=vsb[w * N : (w + 1) * N, h, 0:33],
                                    start=True,
                                    stop=True,
                                    tile_position=(w * N, w * N),
                                )
                        inv = inv_pool.tile([128, H], f32, tag="inv")
                        nc.vector.reciprocal(inv[:], avp[:, :, 32])
                        avsb = av_pool.tile([128, H, 32], dt, tag="av")
                        nc.vector.tensor_mul(
                            avsb[:],
                            avp[:, :, 0:32],
                            inv[:, :, None].broadcast_to([128, H, 32]),
                        )

                        avt = avt_pool.tile([128, 3, 128], dt, tag="avt")
                        late_dma = nc.scalar if dma_split else nc.sync
                        if avt_pe:
                            for cc in range(3):
                                tp = psV.tile([128, 128], dt, tag="avp")
                                nc.tensor.transpose(
                                    tp[:],
                                    avsb[:].rearrange("p h d -> p (h d)")[
                                        :, cc * 128 : (cc + 1) * 128
                                    ],
                                    ident[:],
                                )
                                nc.scalar.copy(avt[:, cc, :], tp[:])
                        else:
                            late_dma.dma_start(
                                out=avt[:],
                                in_=avsb[:].rearrange("p h d -> p (h d)"),
                                transpose=True,
                            )

                        if pipe_tail:
                            pending.append((avt, blk, osb_sup, t0))
                            while len(pending) > 2:
                                flush_pair(pending.pop(0))
                            continue
                        ops = psA.tile([128, DIM], f32, tag="psA")
                        for cc in range(3):
                            nc.tensor.matmul(
                                ops[:],
                                lhsT=avt[:, cc, :],
                                rhs=wp[:, cc, :],
                                start=(cc == 0),
                                stop=(cc == 2),
                            )
                        if v4:
                            nc.vector.tensor_add(osb_sup[:, blk, :], ops[:], pb[:])
                        else:
                            osb = o_pool.tile([128, DIM], f32, tag="osb")
                            nc.vector.tensor_add(osb[:], ops[:], pb[:])
                            tok0 = t0 + blk * PAIR
                            late_dma.dma_start(out_d[tok0 : tok0 + PAIR, :], osb[:])
                if v4 and not pipe_tail:
                    out_ap = out_d[t0 : t0 + SUPER, :].rearrange(
                        "(b p) c -> p b c", p=128
                    )
                    if cast_store:
                        # SWDGE casts bf16 -> f32 during the store
                        nc.gpsimd.dma_start(out_ap, osb_sup[:])
                    else:
                        nc.sync.dma_start(out_ap, osb_sup[:])
            if pipe_tail:
                while pending:
                    flush_pair(pending.pop(0))
    nc.compile()
    return nc


def _build(mode, tok_per_core, reps=1):
    import concourse.bass as bass
    import concourse.mybir as mybir
    import concourse.tile as tile
    from concourse import bacc

    f32 = mybir.dt.float32
    dt = mybir.dt.bfloat16 if mode == "bf16" else f32

    nc = bacc.Bacc("TRN2", target_bir_lowering=False, debug=False)

    x_d = nc.dram_tensor("x", [tok_per_core, DIM], dt, kind="ExternalInput").ap()
    wqk_d = nc.dram_tensor("wqkT", [DIM, 768], dt, kind="ExternalInput").ap()
    wv_d = nc.dram_tensor("wvT", [DIM, DIM], dt, kind="ExternalInput").ap()
    wp_d = nc.dram_tensor("wpT", [DIM, DIM], dt, kind="ExternalInput").ap()
    eb_d = nc.dram_tensor("ebT", [PAIR, 1536], dt, kind="ExternalInput").ap()
    pb_d = nc.dram_tensor("pb", [PAIR, DIM], f32, kind="ExternalInput").ap()
    id_d = nc.dram_tensor("ident", [PAIR, PAIR], f32, kind="ExternalInput").ap()
    out_d = nc.dram_tensor("out", [tok_per_core, DIM], f32, kind="ExternalOutput").ap()

    n_super = tok_per_core // SUPER

    with tile.TileContext(nc) as tc:
        with (
            tc.tile_pool(name="consts", bufs=1) as consts,
            tc.tile_pool(name="xn", bufs=3) as xn_pool,
            tc.tile_pool(name="xt", bufs=3) as xt_pool,
            tc.tile_pool(name="qk", bufs=3) as qk_pool,
            tc.tile_pool(name="v", bufs=3) as v_pool,
            tc.tile_pool(name="er", bufs=4) as er_pool,
            tc.tile_pool(name="et", bufs=4) as et_pool,
            tc.tile_pool(name="av", bufs=4) as av_pool,
            tc.tile_pool(name="avt", bufs=4) as avt_pool,
            tc.tile_pool(name="osb", bufs=4) as o_pool,
            tc.tile_pool(name="inv", bufs=4) as inv_pool,
            tc.tile_pool(name="psA", bufs=2, space="PSUM") as psA,
            tc.tile_pool(name="psS", bufs=4, space="PSUM") as psS,
            tc.tile_pool(name="psB", bufs=2, space="PSUM") as psB,
        ):
            wqk = consts.tile([128, 3, 768], dt)
            nc.sync.dma_start(wqk[:], wqk_d.rearrange("(a p) k -> p a k", p=128))
            wv = consts.tile([128, 3, DIM], dt)
            nc.sync.dma_start(wv[:], wv_d.rearrange("(a p) k -> p a k", p=128))
            wp = consts.tile([128, 3, DIM], dt)
            nc.sync.dma_start(wp[:], wp_d.rearrange("(a p) k -> p a k", p=128))
            eb = consts.tile([PAIR, 1536], dt)
            nc.sync.dma_start(eb[:], eb_d)
            pb = consts.tile([PAIR, DIM], f32)
            nc.sync.dma_start(pb[:], pb_d)
            ident = None
            if mode != "bf16":
                ident = consts.tile([PAIR, PAIR], f32)
                nc.sync.dma_start(ident[:], id_d)
            vbufs = []
            for _i in range(3):
                vper = consts.tile([128, H, 33], dt, tag=f"vper{_i}")
                nc.vector.memset(vper[:, :, 32:33], 1.0)
                vbufs.append(vper)

            for sp in range(n_super * reps):
                t0 = (sp % n_super) * SUPER
                # ---- xT [c, tok] for this super-tile ----
                xt = xt_pool.tile([128, 3, SUPER], dt, tag="xt")
                if mode == "bf16":
                    for cc in range(3):
                        nc.sync.dma_start(
                            out=xt[:, cc, :],
                            in_=x_d[t0 : t0 + SUPER, cc * 128 : (cc + 1) * 128],
                            transpose=True,
                        )
                else:
                    xn = xn_pool.tile([128, 4, DIM], f32, tag="xn")
                    nc.sync.dma_start(
                        xn[:], x_d[t0 : t0 + SUPER, :].rearrange("(b p) c -> p b c", p=128)
                    )
                    for cc in range(3):
                        for tb in range(4):
                            tp = psB.tile([128, 128], f32, tag="bp")
                            nc.tensor.transpose(
                                tp[:], xn[:, tb, cc * 128 : (cc + 1) * 128], ident[:]
                            )
                            nc.scalar.copy(xt[:, cc, tb * 128 : (tb + 1) * 128], tp[:])

                # ---- q,k projections (transposed layout) ----
                qkt = qk_pool.tile([128, 6, SUPER], dt, tag="qkt")
                for j in range(6):
                    ps = psA.tile([128, SUPER], f32, tag="psA")
                    for cc in range(3):
                        nc.tensor.matmul(
                            ps[:],
                            lhsT=wqk[:, cc, j * 128 : (j + 1) * 128],
                            rhs=xt[:, cc, :],
                            start=(cc == 0),
                            stop=(cc == 2),
                        )
                    nc.scalar.copy(qkt[:, j, :], ps[:])

                for blk in range(4):
                    tok0 = t0 + blk * PAIR
                    # ---- v (natural layout, interleaved with ones col) ----
                    vps = psA.tile([128, DIM], f32, tag="psA")
                    for cc in range(3):
                        nc.tensor.matmul(
                            vps[:],
                            lhsT=xt[:, cc, blk * 128 : (blk + 1) * 128],
                            rhs=wv[:, cc, :],
                            start=(cc == 0),
                            stop=(cc == 2),
                        )
                    vsb = vbufs[(sp * 4 + blk) % 3]
                    nc.vector.tensor_copy(
                        vsb[:, :, 0:32], vps[:].rearrange("p (h d) -> p h d", d=32)
                    )

                    # ---- S^T per head: one [32,128]x[32,128] matmul over the
                    # whole pair-tile. Cross-window blocks are garbage; the
                    # bias multiply (eb = 0 there) zeroes them, which makes
                    # E^T block-diagonal so AV is one matmul per head too.
                    # One PSUM bank per PE row-group g=h%4 (concurrent
                    # tile_position matmuls must not share a bank).
                    # Bank g must hold exactly the heads of PE row-group g:
                    # concurrent tile_position matmuls from different row
                    # groups must not write the same PSUM bank.
                    sts = []
                    for _g in range(4):
                        st_g = psS.tile([128, 384], f32, tag="s")
                        sts.append(st_g)
                    for h in range(H):
                        g, j = h % 4, h // 4
                        rp = g * 32
                        f0 = blk * 128
                        nc.tensor.matmul(
                            sts[g][:, j * 128 : (j + 1) * 128],
                            lhsT=qkt[rp : rp + 32, 3 + j, f0 : f0 + 128],
                            rhs=qkt[rp : rp + 32, j, f0 : f0 + 128],
                            start=True,
                            stop=True,
                            tile_position=(rp, 0),
                        )
                    er = er_pool.tile([128, 1536], dt, tag="er")
                    for g in range(4):
                        nc.scalar.activation(
                            er[:, g * 384 : (g + 1) * 384],
                            sts[g][:],
                            mybir.ActivationFunctionType.Exp,
                        )
                    et = et_pool.tile([128, 1536], dt, tag="et")
                    nc.vector.tensor_mul(et[:], er[:], eb[:])

                    # ---- AV (+ denominator in col 32 of each head block) ----
                    avp_t = psB.tile([128, 512], f32, tag="bp")
                    avp = avp_t[:, 0 : H * 33].rearrange("p (h d) -> p h d", d=33)
                    for h in range(H):
                        ec = (h % 4) * 384 + (h // 4) * 128
                        nc.tensor.matmul(
                            avp[:, h, :],
                            lhsT=et[:, ec : ec + 128],
                            rhs=vsb[:, h, :],
                            start=True,
                            stop=True,
                        )
                    inv = inv_pool.tile([128, H], f32, tag="inv")
                    nc.vector.reciprocal(inv[:], avp[:, :, 32])
                    avsb = av_pool.tile([128, H, 32], dt, tag="av")
                    nc.vector.tensor_mul(
                        avsb[:],
                        avp[:, :, 0:32],
                        inv[:, :, None].broadcast_to([128, H, 32]),
                    )

                    # ---- attn^T for the output projection ----
                    avt = avt_pool.tile([128, 3, 128], dt, tag="avt")
                    if mode == "bf16":
                        nc.sync.dma_start(
                            out=avt[:],
                            in_=avsb[:].rearrange("p h d -> p (h d)"),
                            transpose=True,
                        )
                    else:
                        for cc in range(3):
                            tp = psB.tile([128, 128], f32, tag="bp")
                            nc.tensor.transpose(
                                tp[:],
                                avsb[:].rearrange("p h d -> p (h d)")[
                                    :, cc * 128 : (cc + 1) * 128
                                ],
                                ident[:],
                            )
                            nc.scalar.copy(avt[:, cc, :], tp[:])

                    # ---- output projection + bias ----
                    ops = psA.tile([128, DIM], f32, tag="psA")
                    for cc in range(3):
                        nc.tensor.matmul(
                            ops[:],
                            lhsT=avt[:, cc, :],
                            rhs=wp[:, cc, :],
                            start=(cc == 0),
                            stop=(cc == 2),
                        )
                    osb = o_pool.tile([128, DIM], f32, tag="osb")
                    nc.vector.tensor_add(osb[:], ops[:], pb[:])
                    nc.sync.dma_start(out_d[tok0 : tok0 + PAIR, :], osb[:])
    nc.compile()
    return nc


def _build_v7(tok_per_core, reps=1, store_cast=True):
    """v7: v6 + S banks keyed by j (=h//4) so consecutive S matmuls cycle
    PE row-groups (4-way tile_position concurrency) while each bank still
    completes early for exp/mul pipelining; only 3 S banks -> psA gets 3
    bufs. Output projection is software-pipelined 2 pairs behind AV so the
    avt transpose latency hides behind the next group's S work.

    Layout per 2-pair group: stg[j][w*64+k', pr*256 + g*64 + q] holds
    S^T of (pair pr, window w, head h=4j+g).
    """
    import concourse.mybir as mybir
    import concourse.tile as tile
    from concourse import bacc

    f32 = mybir.dt.float32
    dt = mybir.dt.bfloat16

    nc = bacc.Bacc("TRN2", target_bir_lowering=False, debug=False)

    x_d = nc.dram_tensor("x", [tok_per_core, DIM], dt, kind="ExternalInput").ap()
    wqk_d = nc.dram_tensor("wqkT", [DIM, 768], dt, kind="ExternalInput").ap()
    wv_d = nc.dram_tensor("wvT", [DIM, DIM], dt, kind="ExternalInput").ap()
    wp_d = nc.dram_tensor("wpT", [DIM, DIM], dt, kind="ExternalInput").ap()
    eb_d = nc.dram_tensor("ebT", [128, 1536], dt, kind="ExternalInput").ap()
    pb_d = nc.dram_tensor("pb", [PAIR, DIM], f32, kind="ExternalInput").ap()
    id_d = nc.dram_tensor("ident", [PAIR, PAIR], dt, kind="ExternalInput").ap()
    out_d = nc.dram_tensor("out", [tok_per_core, DIM], f32, kind="ExternalOutput").ap()

    n_super = tok_per_core // SUPER

    with tile.TileContext(nc) as tc:
        with (
            tc.tile_pool(name="consts", bufs=1) as consts,
            tc.tile_pool(name="xt", bufs=4) as xt_pool,
            tc.tile_pool(name="qk", bufs=3) as qk_pool,
            tc.tile_pool(name="er", bufs=3) as er_pool,
            tc.tile_pool(name="et", bufs=3) as et_pool,
            tc.tile_pool(name="av", bufs=4) as av_pool,
            tc.tile_pool(name="avt", bufs=6) as avt_pool,
            tc.tile_pool(name="osb", bufs=3) as o_pool,
            tc.tile_pool(name="inv", bufs=4) as inv_pool,
            tc.tile_pool(name="psA", bufs=3, space="PSUM") as psA,
            tc.tile_pool(name="psS", bufs=3, space="PSUM") as psS,
            tc.tile_pool(name="psV", bufs=2, space="PSUM") as psV,
        ):
            wqk = consts.tile([128, 3, 768], dt)
            nc.sync.dma_start(wqk[:], wqk_d.rearrange("(a p) k -> p a k", p=128))
            wv = consts.tile([128, 3, DIM], dt)
            nc.sync.dma_start(wv[:], wv_d.rearrange("(a p) k -> p a k", p=128))
            wp = consts.tile([128, 3, DIM], dt)
            nc.sync.dma_start(wp[:], wp_d.rearrange("(a p) k -> p a k", p=128))
            eb2 = consts.tile([128, 3, 512], dt)
            nc.sync.dma_start(eb2[:], eb_d.rearrange("p (a k) -> p a k", a=3))
            vbufs = []
            for _i in range(6):
                vper = consts.tile([128, H, 33], dt, tag=f"vper{_i}")
                nc.vector.memset(vper[:, :, 32:33], 1.0)
                vbufs.append(vper)

            pending = []

            def flush_pair(item):
                avt, blk, osb_t, pt0 = item
                ops = psA.tile([128, DIM], f32, tag="psA")
                for cc in range(3):
                    nc.tensor.matmul(
                        ops[:],
                        lhsT=avt[:, cc, :],
                        rhs=wp[:, cc, :],
                        start=(cc == 0),
                        stop=(cc == 2),
                    )
                nc.vector.tensor_copy(osb_t[:, blk, :], ops[:])
                if blk == 3:
                    out_ap = out_d[pt0 : pt0 + SUPER, :].rearrange(
                        "(b p) c -> p b c", p=128
                    )
                    if store_cast:
                        nc.gpsimd.dma_start(out_ap, osb_t[:])
                    else:
                        nc.sync.dma_start(out_ap, osb_t[:])

            for sp in range(n_super * reps):
                t0 = (sp % n_super) * SUPER
                xt = xt_pool.tile([128, 3, SUPER], dt, tag="xt")
                for cc in range(3):
                    nc.sync.dma_start(
                        out=xt[:, cc, :],
                        in_=x_d[t0 : t0 + SUPER, cc * 128 : (cc + 1) * 128],
                        transpose=True,
                    )

                qkt = qk_pool.tile([128, 6, SUPER], dt, tag="qkt")
                for j in range(6):
                    ps = psA.tile([128, SUPER], f32, tag="psA")
                    for cc in range(3):
                        nc.tensor.matmul(
                            ps[:],
                            lhsT=wqk[:, cc, j * 128 : (j + 1) * 128],
                            rhs=xt[:, cc, :],
                            start=(cc == 0),
                            stop=(cc == 2),
                        )
                    nc.scalar.copy(qkt[:, j, :], ps[:])

                for blk in range(4):
                    vps = psA.tile([128, DIM], f32, tag="psA")
                    for cc in range(3):
                        nc.tensor.matmul(
                            vps[:],
                            lhsT=xt[:, cc, blk * 128 : (blk + 1) * 128],
                            rhs=wv[:, cc, :],
                            start=(cc == 0),
                            stop=(cc == 2),
                        )
                    vsb = vbufs[(sp * 4 + blk) % 6]
                    nc.vector.tensor_copy(
                        vsb[:, :, 0:32], vps[:].rearrange("p (h d) -> p h d", d=32)
                    )
                osb_sup = o_pool.tile([128, 4, DIM], dt if store_cast else f32,
                                      tag="osb")

                for half in range(2):
                    er2 = er_pool.tile([128, 3, 512], dt, tag="er")
                    et2 = et_pool.tile([128, 3, 512], dt, tag="et")
                    for j in range(3):
                        st_j = psS.tile([128, 512], f32, tag="s")
                        for pr in range(2):
                            blk = 2 * half + pr
                            for g in range(4):
                                rp = g * 32
                                for w in range(2):
                                    f0 = blk * PAIR + w * N
                                    c0 = pr * 256 + g * 64
                                    nc.tensor.matmul(
                                        st_j[w * N : (w + 1) * N, c0 : c0 + 64],
                                        lhsT=qkt[rp : rp + 32, 3 + j, f0 : f0 + N],
                                        rhs=qkt[rp : rp + 32, j, f0 : f0 + N],
                                        start=True,
                                        stop=True,
                                        tile_position=(rp, w * N),
                                    )
                        nc.scalar.activation(
                            er2[:, j, :], st_j[:], mybir.ActivationFunctionType.Exp
                        )
                        nc.vector.tensor_mul(et2[:, j, :], er2[:, j, :], eb2[:, j, :])

                    for pr in range(2):
                        blk = 2 * half + pr
                        vsb = vbufs[(sp * 4 + blk) % 6]
                        avp_t = psV.tile([128, H * 33], f32, tag="avp")
                        avp = avp_t[:].rearrange("p (h d) -> p h d", d=33)
                        for h in range(H):
                            g, j = h % 4, h // 4
                            c0 = pr * 256 + g * 64
                            for w in range(2):
                                nc.tensor.matmul(
                                    avp[w * N : (w + 1) * N, h, :],
                                    lhsT=et2[w * N : (w + 1) * N, j, c0 : c0 + 64],
                                    rhs=vsb[w * N : (w + 1) * N, h, 0:33],
                                    start=True,
                                    stop=True,
                                    tile_position=(w * N, w * N),
                                )
                        inv = inv_pool.tile([128, H], f32, tag="inv")
                        nc.vector.reciprocal(inv[:], avp[:, :, 32])
                        avsb = av_pool.tile([128, H, 32], dt, tag="av")
                        nc.vector.tensor_mul(
                            avsb[:],
                            avp[:, :, 0:32],
                            inv[:, :, None].broadcast_to([128, H, 32]),
                        )
                        avt = avt_pool.tile([128, 3, 128], dt, tag="avt")
                        nc.scalar.dma_start(
                            out=avt[:],
                            in_=avsb[:].rearrange("p h d -> p (h d)"),
                            transpose=True,
                        )
                        pending.append((avt, blk, osb_sup, t0))
                        while len(pending) > 2:
                            flush_pair(pending.pop(0))
            while pending:
                flush_pair(pending.pop(0))
    nc.compile()
    return nc


def _get_nc(mode, tok_per_core, reps=1):
    key = (mode, tok_per_core, reps)
    if key not in _cache:
        if mode in ("v7", "v7s"):
            _cache[key] = _build_v7(tok_per_core, reps, store_cast=(mode == "v7"))
        elif mode.startswith(("v2", "v3", "v4", "v5", "v6", "v8")):
            _cache[key] = _build_v2(
                tok_per_core,
                reps,
                avt_pe=mode.endswith("pe"),
                dma_split=not mode.startswith("v2"),
                v4=mode.startswith(("v4", "v5", "v6", "v8")),
                bank_major=mode.startswith(("v5", "v6")),
                pool_mul=(mode == "v5p"),
                v6=mode.startswith(("v6", "v8")),
                pipe_tail=mode.startswith("v8"),
                cast_store=False if mode == "v8f" else None,
            )
        else:
            _cache[key] = _build(mode, tok_per_core, reps)
    return _cache[key]


def _host_prep(x, qkv_w, proj_w, proj_b, bias_table, rel_idx, mode, n_cores):
    np_dt = np.float32 if mode == "f32" else ml_dtypes.bfloat16
    x = np.asarray(x, np.float32)
    qkv_w = np.asarray(qkv_w, np.float32)
    proj_w = np.asarray(proj_w, np.float32)
    proj_b = np.asarray(proj_b, np.float32)
    bias_table = np.asarray(bias_table, np.float32)
    rel_idx = np.asarray(rel_idx)

    wq = qkv_w[0:DIM] * SCALE
    wk = qkv_w[DIM : 2 * DIM]
    wv = qkv_w[2 * DIM :]
    wqkT = np.concatenate([wq, wk], 0).T.copy().astype(np_dt)  # [384, 768]
    wvT = wv.T.copy().astype(np_dt)
    wpT = proj_w.T.copy().astype(np_dt)

    bias = bias_table[rel_idx.reshape(-1)].reshape(N, N, H)  # [nq, nk, h]
    eb1 = np.exp(bias).transpose(1, 2, 0)  # [nk, h, nq]
    if mode.startswith("v7"):
        # eb2 [128, (j, pr, g, q)]: rows repeat at 64 (same for both windows)
        ebT = np.zeros((PAIR, 3, 2, 4, N), np.float32)
        for h in range(H):
            g, j = h % 4, h // 4
            for w in range(2):
                for pr in range(2):
                    ebT[w * N : (w + 1) * N, j, pr, g, :] = eb1[:, h, :]
        ebT = ebT.reshape(PAIR, H * PAIR).astype(np_dt)
        ident = np.eye(PAIR, dtype=np_dt)
    elif mode.startswith(("v2", "v3", "v4", "v5", "v6", "v8")):
        # eb2 [128, (g, pr, j, q)]: rows repeat at 64 (same for both windows)
        ebT = np.zeros((PAIR, 4, 2, 3, N), np.float32)
        for h in range(H):
            g, j = h % 4, h // 4
            for w in range(2):
                for pr in range(2):
                    ebT[w * N : (w + 1) * N, g, pr, j, :] = eb1[:, h, :]
        ebT = ebT.reshape(PAIR, H * PAIR).astype(np_dt)
        ident = np.eye(PAIR, dtype=np_dt)
    else:
        ebT = np.zeros((PAIR, H * PAIR), np.float32)  # cross-window blocks stay 0
        for h in range(H):
            ec = (h % 4) * 384 + (h // 4) * 128
            for w in range(2):
                ebT[w * N : (w + 1) * N, ec + w * N : ec + (w + 1) * N] = eb1[:, h, :]
        ebT = ebT.astype(np_dt)  # [128, 1536]
        ident = np.eye(PAIR, dtype=np.float32)
    pb = np.broadcast_to(proj_b, (PAIR, DIM)).copy().astype(np.float32)

    B = x.shape[0]
    bper = B // n_cores
    xs = x.reshape(B * N, DIM).astype(np_dt)
    in_maps = []
    for c in range(n_cores):
        in_maps.append(
            {
                "x": xs[c * bper * N : (c + 1) * bper * N],
                "wqkT": wqkT,
                "wvT": wvT,
                "wpT": wpT,
                "ebT": ebT,
                "pb": pb,
                "ident": ident,
            }
        )
    return in_maps


def kernel(x, qkv_w, proj_w, proj_b, bias_table, rel_idx):
    from concourse.bass_utils import run_bass_kernel_spmd

    x = np.asarray(x)
    B = x.shape[0]
    n_cores = NCORES
    tok_per_core = (B // n_cores) * N
    nc = _get_nc(MODE, tok_per_core)
    in_maps = _host_prep(x, qkv_w, proj_w, proj_b, bias_table, rel_idx, MODE, n_cores)
    res = run_bass_kernel_spmd(nc, in_maps, list(range(n_cores)))
    out = np.concatenate([r["out"] for r in res.results], 0)
    return out.reshape(B, N, DIM).astype(np.float32)



# revision 62
# speedup vs baseline: 1.3415x; 1.1127x over previous
"""BoxAttention TRN2 kernel — 8-core data-parallel over the window dim.

Per core: 256 windows x 64 tokens x 384 dim, 12 heads, head_dim 32.
Default mode "v4" (bf16), per 512-token super-tile (8 windows):

  xT  (c,tok)   <- 3 HBM DMA-transposes (SP HWDGE ring, prefetched)
  qT,kT         <- W_qk^T stationary matmuls, rhs = xT; PSUM->SBUF on ACT
  v   (tok,h,d) <- all 4 pair-tiles hoisted up front (releases xT early
                   for next-super prefetch); ones col 32 = denominator
  S^T           <- per (pair, window, head): [32,64]x[32,64] quadrant
                   matmuls via tile_position; w0 -> PSUM partitions 0:64,
                   w1 -> 64:128 of row-group bank g=h%4 (no cross-window
                   garbage blocks, 8-way PE-array concurrency)
  E^T           <- exp(S^T) batched over 2 pairs per bank (ACT), then
                   * exp(bias) (DVE; bias folded via exp, no add needed)
  AV            <- per (window, head) quadrant matmuls; col 32 = denom
  attn (tok,c)  <- AV * 1/denom (DVE, free-dim broadcast)
  attn^T        <- SBUF DMA-transpose on the scalar HWDGE ring
  out (tok,o)   <- attn^T stationary matmuls + proj_b; one batched
                   768 KB store per super-tile

The two HWDGE rings are split (SP: prefetch-side, ACT: result-side) so
late-pipeline DMAs never head-of-line block next-super xT prefetches.
Measured 0.628 ms/rep per core on HW (8 cores run the 2048-window batch
data-parallel, no cross-core communication).
"""

import os
import sys
import numpy as np

for _p in ("/opt/trn_rl_repo", "/opt/pypackages"):
    if _p not in sys.path and os.path.isdir(_p):
        sys.path.append(_p)

import ml_dtypes  # noqa: E402

DIM, BOX, H = 384, 4, 12
N = BOX ** 3            # 64 tokens per window
HD = DIM // H           # 32
SCALE = HD ** -0.5
B_ = 2048
NCORES = 8
B_PER = B_ // NCORES    # 256 windows per core
TOK = B_PER * N         # 16384 tokens per core
SUPER = 512             # tokens per super-tile (8 windows)
PAIR = 128              # tokens per pair-tile (2 windows)

# "v4" (default): bf16, window-split S/AV with tile_position quadrant packing,
# hoisted v-projections, per-super batched output stores, HWDGE ring split.
# Other modes ("f32", "bf16", "v2", ...) kept for A/B debugging.
MODE = os.environ.get("BOXATTN_MODE", "v4")

_cache = {}


def _build_v2(tok_per_core, reps=1, avt_pe=False, dma_split=False, v4=False,
              bank_major=False, pool_mul=False, v6=False, pipe_tail=False,
              cast_store=None):
    """Window-split bf16 kernel: S/AV computed per (window, head) with
    tile_position quadrant packing — no cross-window garbage blocks, so
    exp/bias-mul/AV contraction are half the work of the pair-packed
    layout. exp is batched over 2 pair-tiles (one [128,384] ACT op per
    PSUM row-group bank).

    Layouts per 2-pair group (pairs pr=0,1; windows w=0,1; head h=(g,j)
    with g=h%4, j=h//4):
      stg[g][64w:64w+64, pr*192+j*64+q] = S^T[k, q] of (pair pr, w, h)
      et2  [128, (g, pr*192+j*64+q)]    = exp(S^T) * exp(bias)
      avp  [tok128, h, 0:33]            = unnormalized AV | denominator
    """
    import concourse.bass as bass
    import concourse.mybir as mybir
    import concourse.tile as tile
    from concourse import bacc

    f32 = mybir.dt.float32
    dt = mybir.dt.bfloat16

    nc = bacc.Bacc("TRN2", target_bir_lowering=False, debug=False)

    x_d = nc.dram_tensor("x", [tok_per_core, DIM], dt, kind="ExternalInput").ap()
    wqk_d = nc.dram_tensor("wqkT", [DIM, 768], dt, kind="ExternalInput").ap()
    wv_d = nc.dram_tensor("wvT", [DIM, DIM], dt, kind="ExternalInput").ap()
    wp_d = nc.dram_tensor("wpT", [DIM, DIM], dt, kind="ExternalInput").ap()
    eb_d = nc.dram_tensor("ebT", [128, 1536], dt, kind="ExternalInput").ap()
    pb_d = nc.dram_tensor("pb", [PAIR, DIM], f32, kind="ExternalInput").ap()
    id_d = nc.dram_tensor("ident", [PAIR, PAIR], dt, kind="ExternalInput").ap()
    out_d = nc.dram_tensor("out", [tok_per_core, DIM], f32, kind="ExternalOutput").ap()

    if cast_store is None:
        cast_store = v6
    n_super = tok_per_core // SUPER

    with tile.TileContext(nc) as tc:
        with (
            tc.tile_pool(name="consts", bufs=1) as consts,
            tc.tile_pool(name="xt", bufs=4 if v4 else 3) as xt_pool,
            tc.tile_pool(name="qk", bufs=3) as qk_pool,
            tc.tile_pool(name="er", bufs=3) as er_pool,
            tc.tile_pool(name="et", bufs=3) as et_pool,
            tc.tile_pool(name="av", bufs=4) as av_pool,
            tc.tile_pool(name="avt", bufs=4) as avt_pool,
            tc.tile_pool(name="osb", bufs=4) as o_pool,
            tc.tile_pool(name="inv", bufs=4) as inv_pool,
            tc.tile_pool(name="psA", bufs=2, space="PSUM") as psA,
            tc.tile_pool(name="psS", bufs=4, space="PSUM") as psS,
            tc.tile_pool(name="psV", bufs=2, space="PSUM") as psV,
        ):
            wqk = consts.tile([128, 3, 768], dt)
            nc.sync.dma_start(wqk[:], wqk_d.rearrange("(a p) k -> p a k", p=128))
            wv = consts.tile([128, 3, DIM], dt)
            nc.sync.dma_start(wv[:], wv_d.rearrange("(a p) k -> p a k", p=128))
            wp = consts.tile([128, 3, DIM], dt)
            nc.sync.dma_start(wp[:], wp_d.rearrange("(a p) k -> p a k", p=128))
            eb2 = consts.tile([128, 4, 384], dt)
            nc.sync.dma_start(eb2[:], eb_d.rearrange("p (a k) -> p a k", a=4))
            pb = consts.tile([PAIR, DIM], f32)
            nc.sync.dma_start(pb[:], pb_d)

            pending = []

            def flush_pair(item):
                f_avt, f_blk, f_osb, f_t0 = item
                ops = psA.tile([128, DIM], f32, tag="psA")
                for cc in range(3):
                    nc.tensor.matmul(
                        ops[:],
                        lhsT=f_avt[:, cc, :],
                        rhs=wp[:, cc, :],
                        start=(cc == 0),
                        stop=(cc == 2),
                    )
                nc.vector.tensor_add(f_osb[:, f_blk, :], ops[:], pb[:])
                if f_blk == 3:
                    out_ap = out_d[f_t0 : f_t0 + SUPER, :].rearrange(
                        "(b p) c -> p b c", p=128
                    )
                    if cast_store:
                        nc.gpsimd.dma_start(out_ap, f_osb[:])
                    else:
                        nc.sync.dma_start(out_ap, f_osb[:])
            ident = None
            if avt_pe:
                ident = consts.tile([PAIR, PAIR], dt)
                nc.sync.dma_start(ident[:], id_d)
            nvbuf = 6 if v4 else 3
            vbufs = []
            for _i in range(nvbuf):
                vper = consts.tile([128, H, 33], dt, tag=f"vper{_i}")
                nc.vector.memset(vper[:, :, 32:33], 1.0)
                vbufs.append(vper)

            for sp in range(n_super * reps):
                t0 = (sp % n_super) * SUPER
                xt = xt_pool.tile([128, 3, SUPER], dt, tag="xt")
                for cc in range(3):
                    nc.sync.dma_start(
                        out=xt[:, cc, :],
                        in_=x_d[t0 : t0 + SUPER, cc * 128 : (cc + 1) * 128],
                        transpose=True,
                    )

                qkt = qk_pool.tile([128, 6, SUPER], dt, tag="qkt")
                for j in range(6):
                    ps = psA.tile([128, SUPER], f32, tag="psA")
                    for cc in range(3):
                        nc.tensor.matmul(
                            ps[:],
                            lhsT=wqk[:, cc, j * 128 : (j + 1) * 128],
                            rhs=xt[:, cc, :],
                            start=(cc == 0),
                            stop=(cc == 2),
                        )
                    nc.scalar.copy(qkt[:, j, :], ps[:])

                if v4:
                    # hoist all v projections: releases xt for next-super
                    # prefetch half a super earlier
                    for blk in range(4):
                        vps = psA.tile([128, DIM], f32, tag="psA")
                        for cc in range(3):
                            nc.tensor.matmul(
                                vps[:],
                                lhsT=xt[:, cc, blk * 128 : (blk + 1) * 128],
                                rhs=wv[:, cc, :],
                                start=(cc == 0),
                                stop=(cc == 2),
                            )
                        vsb = vbufs[(sp * 4 + blk) % nvbuf]
                        nc.vector.tensor_copy(
                            vsb[:, :, 0:32], vps[:].rearrange("p (h d) -> p h d", d=32)
                        )
                    osb_sup = o_pool.tile([128, 4, DIM], dt if cast_store else f32,
                                          tag="osb")

                for half in range(2):
                    stg = []
                    for _g in range(4):
                        st_g = psS.tile([128, 384], f32, tag="s")
                        stg.append(st_g)
                    if bank_major:
                        # Emit S matmuls bank-major and fire exp(g)+mul(g) as
                        # soon as bank g completes, so ACT/DVE overlap the
                        # remaining banks' matmuls.
                        er2 = er_pool.tile([128, 4, 384], dt, tag="er")
                        et2 = et_pool.tile([128, 4, 384], dt, tag="et")
                        for g in range(4):
                            rp = g * 32
                            for pr in range(2):
                                blk = 2 * half + pr
                                for j in range(3):
                                    h = 4 * j + g
                                    for w in range(2):
                                        f0 = blk * PAIR + w * N
                                        c0 = pr * 192 + j * 64
                                        nc.tensor.matmul(
                                            stg[g][w * N : (w + 1) * N, c0 : c0 + 64],
                                            lhsT=qkt[rp : rp + 32, 3 + j, f0 : f0 + N],
                                            rhs=qkt[rp : rp + 32, j, f0 : f0 + N],
                                            start=True,
                                            stop=True,
                                            tile_position=(rp, w * N),
                                        )
                            nc.scalar.activation(
                                er2[:, g, :], stg[g][:],
                                mybir.ActivationFunctionType.Exp,
                            )
                            mul_eng = nc.gpsimd if pool_mul else nc.vector
                            mul_eng.tensor_mul(
                                et2[:, g, :], er2[:, g, :], eb2[:, g, :]
                            )
                    for pr in range(2):
                        blk = 2 * half + pr
                        if not v4:
                            # ---- v (natural layout, ones col for denom) ----
                            vps = psA.tile([128, DIM], f32, tag="psA")
                            for cc in range(3):
                                nc.tensor.matmul(
                                    vps[:],
                                    lhsT=xt[:, cc, blk * 128 : (blk + 1) * 128],
                                    rhs=wv[:, cc, :],
                                    start=(cc == 0),
                                    stop=(cc == 2),
                                )
                            vsb = vbufs[(sp * 4 + blk) % nvbuf]
                            nc.vector.tensor_copy(
                                vsb[:, :, 0:32],
                                vps[:].rearrange("p (h d) -> p h d", d=32),
                            )
                        # ---- S^T per (window, head): [32,64]x[32,64] in a
                        # quadrant; w0 -> rows/psum-partitions 0:64, w1 ->
                        # 64:128. Different windows write disjoint partition
                        # halves of the same bank, so they can overlap.
                        if not bank_major:
                            for h in range(H):
                                g, j = h % 4, h // 4
                                rp = g * 32
                                for w in range(2):
                                    f0 = blk * PAIR + w * N
                                    c0 = pr * 192 + j * 64
                                    nc.tensor.matmul(
                                        stg[g][w * N : (w + 1) * N, c0 : c0 + 64],
                                        lhsT=qkt[rp : rp + 32, 3 + j, f0 : f0 + N],
                                        rhs=qkt[rp : rp + 32, j, f0 : f0 + N],
                                        start=True,
                                        stop=True,
                                        tile_position=(rp, w * N),
                                    )
                    if not bank_major:
                        # ---- exp over both pairs, one ACT op per bank ----
                        er2 = er_pool.tile([128, 4, 384], dt, tag="er")
                        for g in range(4):
                            nc.scalar.activation(
                                er2[:, g, :], stg[g][:],
                                mybir.ActivationFunctionType.Exp,
                            )
                        et2 = et_pool.tile([128, 4, 384], dt, tag="et")
                        nc.vector.tensor_mul(et2[:], er2[:], eb2[:])

                    for pr in range(2):
                        blk = 2 * half + pr
                        vsb = vbufs[(sp * 4 + blk) % nvbuf]
                        avp_t = psV.tile([128, H * 33], f32, tag="avp")
                        avp = avp_t[:].rearrange("p (h d) -> p h d", d=33)
                        for h in range(H):
                            g, j = h % 4, h // 4
                            c0 = pr * 192 + j * 64
                            for w in range(2):
                                nc.tensor.matmul(
                                    avp[w * N : (w + 1) * N, h, :],
                                    lhsT=et2[w * N : (w + 1) * N, g, c0 : c0 + 64],
                                    rhs=vsb[w * N : (w + 1) * N, h, 0:33],
                                    start=True,
                                    stop=True,
                                    tile_position=(w * N, w * N),
                                )
                        inv = inv_pool.tile([128, H], f32, tag="inv")
                        nc.vector.reciprocal(inv[:], avp[:, :, 32])
                        avsb = av_pool.tile([128, H, 32], dt, tag="av")
                        nc.vector.tensor_mul(
                            avsb[:],
                            avp[:, :, 0:32],
                            inv[:, :, None].broadcast_to([128, H, 32]),
                        )

                        avt = avt_pool.tile([128, 3, 128], dt, tag="avt")
                        late_dma = nc.scalar if dma_split else nc.sync
                        if avt_pe:
                            for cc in range(3):
                                tp = psV.tile([128, 128], dt, tag="avp")
                                nc.tensor.transpose(
                                    tp[:],
                                    avsb[:].rearrange("p h d -> p (h d)")[
                                        :, cc * 128 : (cc + 1) * 128
                                    ],
                                    ident[:],
                                )
                                nc.scalar.copy(avt[:, cc, :], tp[:])
                        else:
                            late_dma.dma_start(
                                out=avt[:],
                                in_=avsb[:].rearrange("p h d -> p (h d)"),
                                transpose=True,
                            )

                        if pipe_tail:
                            pending.append((avt, blk, osb_sup, t0))
                            while len(pending) > 2:
                                flush_pair(pending.pop(0))
                            continue
                        ops = psA.tile([128, DIM], f32, tag="psA")
                        for cc in range(3):
                            nc.tensor.matmul(
                                ops[:],
                                lhsT=avt[:, cc, :],
                                rhs=wp[:, cc, :],
                                start=(cc == 0),
                                stop=(cc == 2),
                            )
                        if v4:
                            nc.vector.tensor_add(osb_sup[:, blk, :], ops[:], pb[:])
                        else:
                            osb = o_pool.tile([128, DIM], f32, tag="osb")
                            nc.vector.tensor_add(osb[:], ops[:], pb[:])
                            tok0 = t0 + blk * PAIR
                            late_dma.dma_start(out_d[tok0 : tok0 + PAIR, :], osb[:])
                if v4 and not pipe_tail:
                    out_ap = out_d[t0 : t0 + SUPER, :].rearrange(
                        "(b p) c -> p b c", p=128
                    )
                    if cast_store:
                        # SWDGE casts bf16 -> f32 during the store
                        nc.gpsimd.dma_start(out_ap, osb_sup[:])
                    else:
                        nc.sync.dma_start(out_ap, osb_sup[:])
            if pipe_tail:
                while pending:
                    flush_pair(pending.pop(0))
    nc.compile()
    return nc


def _build(mode, tok_per_core, reps=1):
    import concourse.bass as bass
    import concourse.mybir as mybir
    import concourse.tile as tile
    from concourse import bacc

    f32 = mybir.dt.float32
    dt = mybir.dt.bfloat16 if mode == "bf16" else f32

    nc = bacc.Bacc("TRN2", target_bir_lowering=False, debug=False)

    x_d = nc.dram_tensor("x", [tok_per_core, DIM], dt, kind="ExternalInput").ap()
    wqk_d = nc.dram_tensor("wqkT", [DIM, 768], dt, kind="ExternalInput").ap()
    wv_d = nc.dram_tensor("wvT", [DIM, DIM], dt, kind="ExternalInput").ap()
    wp_d = nc.dram_tensor("wpT", [DIM, DIM], dt, kind="ExternalInput").ap()
    eb_d = nc.dram_tensor("ebT", [PAIR, 1536], dt, kind="ExternalInput").ap()
    pb_d = nc.dram_tensor("pb", [PAIR, DIM], f32, kind="ExternalInput").ap()
    id_d = nc.dram_tensor("ident", [PAIR, PAIR], f32, kind="ExternalInput").ap()
    out_d = nc.dram_tensor("out", [tok_per_core, DIM], f32, kind="ExternalOutput").ap()

    n_super = tok_per_core // SUPER

    with tile.TileContext(nc) as tc:
        with (
            tc.tile_pool(name="consts", bufs=1) as consts,
            tc.tile_pool(name="xn", bufs=3) as xn_pool,
            tc.tile_pool(name="xt", bufs=3) as xt_pool,
            tc.tile_pool(name="qk", bufs=3) as qk_pool,
            tc.tile_pool(name="v", bufs=3) as v_pool,
            tc.tile_pool(name="er", bufs=4) as er_pool,
            tc.tile_pool(name="et", bufs=4) as et_pool,
            tc.tile_pool(name="av", bufs=4) as av_pool,
            tc.tile_pool(name="avt", bufs=4) as avt_pool,
            tc.tile_pool(name="osb", bufs=4) as o_pool,
            tc.tile_pool(name="inv", bufs=4) as inv_pool,
            tc.tile_pool(name="psA", bufs=2, space="PSUM") as psA,
            tc.tile_pool(name="psS", bufs=4, space="PSUM") as psS,
            tc.tile_pool(name="psB", bufs=2, space="PSUM") as psB,
        ):
            wqk = consts.tile([128, 3, 768], dt)
            nc.sync.dma_start(wqk[:], wqk_d.rearrange("(a p) k -> p a k", p=128))
            wv = consts.tile([128, 3, DIM], dt)
            nc.sync.dma_start(wv[:], wv_d.rearrange("(a p) k -> p a k", p=128))
            wp = consts.tile([128, 3, DIM], dt)
            nc.sync.dma_start(wp[:], wp_d.rearrange("(a p) k -> p a k", p=128))
            eb = consts.tile([PAIR, 1536], dt)
            nc.sync.dma_start(eb[:], eb_d)
            pb = consts.tile([PAIR, DIM], f32)
            nc.sync.dma_start(pb[:], pb_d)
            ident = None
            if mode != "bf16":
                ident = consts.tile([PAIR, PAIR], f32)
                nc.sync.dma_start(ident[:], id_d)
            vbufs = []
            for _i in range(3):
                vper = consts.tile([128, H, 33], dt, tag=f"vper{_i}")
                nc.vector.memset(vper[:, :, 32:33], 1.0)
                vbufs.append(vper)

            for sp in range(n_super * reps):
                t0 = (sp % n_super) * SUPER
                # ---- xT [c, tok] for this super-tile ----
                xt = xt_pool.tile([128, 3, SUPER], dt, tag="xt")
                if mode == "bf16":
                    for cc in range(3):
                        nc.sync.dma_start(
                            out=xt[:, cc, :],
                            in_=x_d[t0 : t0 + SUPER, cc * 128 : (cc + 1) * 128],
                            transpose=True,
                        )
                else:
                    xn = xn_pool.tile([128, 4, DIM], f32, tag="xn")
                    nc.sync.dma_start(
                        xn[:], x_d[t0 : t0 + SUPER, :].rearrange("(b p) c -> p b c", p=128)
                    )
                    for cc in range(3):
                        for tb in range(4):
                            tp = psB.tile([128, 128], f32, tag="bp")
                            nc.tensor.transpose(
                                tp[:], xn[:, tb, cc * 128 : (cc + 1) * 128], ident[:]
                            )
                            nc.scalar.copy(xt[:, cc, tb * 128 : (tb + 1) * 128], tp[:])

                # ---- q,k projections (transposed layout) ----
                qkt = qk_pool.tile([128, 6, SUPER], dt, tag="qkt")
                for j in range(6):
                    ps = psA.tile([128, SUPER], f32, tag="psA")
                    for cc in range(3):
                        nc.tensor.matmul(
                            ps[:],
                            lhsT=wqk[:, cc, j * 128 : (j + 1) * 128],
                            rhs=xt[:, cc, :],
                            start=(cc == 0),
                            stop=(cc == 2),
                        )
                    nc.scalar.copy(qkt[:, j, :], ps[:])

                for blk in range(4):
                    tok0 = t0 + blk * PAIR
                    # ---- v (natural layout, interleaved with ones col) ----
                    vps = psA.tile([128, DIM], f32, tag="psA")
                    for cc in range(3):
                        nc.tensor.matmul(
                            vps[:],
                            lhsT=xt[:, cc, blk * 128 : (blk + 1) * 128],
                            rhs=wv[:, cc, :],
                            start=(cc == 0),
                            stop=(cc == 2),
                        )
                    vsb = vbufs[(sp * 4 + blk) % 3]
                    nc.vector.tensor_copy(
                        vsb[:, :, 0:32], vps[:].rearrange("p (h d) -> p h d", d=32)
                    )

                    # ---- S^T per head: one [32,128]x[32,128] matmul over the
                    # whole pair-tile. Cross-window blocks are garbage; the
                    # bias multiply (eb = 0 there) zeroes them, which makes
                    # E^T block-diagonal so AV is one matmul per head too.
                    # One PSUM bank per PE row-group g=h%4 (concurrent
                    # tile_position matmuls must not share a bank).
                    # Bank g must hold exactly the heads of PE row-group g:
                    # concurrent tile_position matmuls from different row
                    # groups must not write the same PSUM bank.
                    sts = []
                    for _g in range(4):
                        st_g = psS.tile([128, 384], f32, tag="s")
                        sts.append(st_g)
                    for h in range(H):
                        g, j = h % 4, h // 4
                        rp = g * 32
                        f0 = blk * 128
                        nc.tensor.matmul(
                            sts[g][:, j * 128 : (j + 1) * 128],
                            lhsT=qkt[rp : rp + 32, 3 + j, f0 : f0 + 128],
                            rhs=qkt[rp : rp + 32, j, f0 : f0 + 128],
                            start=True,
                            stop=True,
                            tile_position=(rp, 0),
                        )
                    er = er_pool.tile([128, 1536], dt, tag="er")
                    for g in range(4):
                        nc.scalar.activation(
                            er[:, g * 384 : (g + 1) * 384],
                            sts[g][:],
                            mybir.ActivationFunctionType.Exp,
                        )
                    et = et_pool.tile([128, 1536], dt, tag="et")
                    nc.vector.tensor_mul(et[:], er[:], eb[:])

                    # ---- AV (+ denominator in col 32 of each head block) ----
                    avp_t = psB.tile([128, 512], f32, tag="bp")
                    avp = avp_t[:, 0 : H * 33].rearrange("p (h d) -> p h d", d=33)
                    for h in range(H):
                        ec = (h % 4) * 384 + (h // 4) * 128
                        nc.tensor.matmul(
                            avp[:, h, :],
                            lhsT=et[:, ec : ec + 128],
                            rhs=vsb[:, h, :],
                            start=True,
                            stop=True,
                        )
                    inv = inv_pool.tile([128, H], f32, tag="inv")
                    nc.vector.reciprocal(inv[:], avp[:, :, 32])
                    avsb = av_pool.tile([128, H, 32], dt, tag="av")
                    nc.vector.tensor_mul(
                        avsb[:],
                        avp[:, :, 0:32],
                        inv[:, :, None].broadcast_to([128, H, 32]),
                    )

                    # ---- attn^T for the output projection ----
                    avt = avt_pool.tile([128, 3, 128], dt, tag="avt")
                    if mode == "bf16":
                        nc.sync.dma_start(
                            out=avt[:],
                            in_=avsb[:].rearrange("p h d -> p (h d)"),
                            transpose=True,
                        )
                    else:
                        for cc in range(3):
                            tp = psB.tile([128, 128], f32, tag="bp")
                            nc.tensor.transpose(
                                tp[:],
                                avsb[:].rearrange("p h d -> p (h d)")[
                                    :, cc * 128 : (cc + 1) * 128
                                ],
                                ident[:],
                            )
                            nc.scalar.copy(avt[:, cc, :], tp[:])

                    # ---- output projection + bias ----
                    ops = psA.tile([128, DIM], f32, tag="psA")
                    for cc in range(3):
                        nc.tensor.matmul(
                            ops[:],
                            lhsT=avt[:, cc, :],
                            rhs=wp[:, cc, :],
                            start=(cc == 0),
                            stop=(cc == 2),
                        )
                    osb = o_pool.tile([128, DIM], f32, tag="osb")
                    nc.vector.tensor_add(osb[:], ops[:], pb[:])
                    nc.sync.dma_start(out_d[tok0 : tok0 + PAIR, :], osb[:])
    nc.compile()
    return nc


def _build_v7(tok_per_core, reps=1, store_cast=True):
    """v7: v6 + S banks keyed by j (=h//4) so consecutive S matmuls cycle
    PE row-groups (4-way tile_position concurrency) while each bank still
    completes early for exp/mul pipelining; only 3 S banks -> psA gets 3
    bufs. Output projection is software-pipelined 2 pairs behind AV so the
    avt transpose latency hides behind the next group's S work.

    Layout per 2-pair group: stg[j][w*64+k', pr*256 + g*64 + q] holds
    S^T of (pair pr, window w, head h=4j+g).
    """
    import concourse.mybir as mybir
    import concourse.tile as tile
    from concourse import bacc

    f32 = mybir.dt.float32
    dt = mybir.dt.bfloat16

    nc = bacc.Bacc("TRN2", target_bir_lowering=False, debug=False)

    x_d = nc.dram_tensor("x", [tok_per_core, DIM], dt, kind="ExternalInput").ap()
    wqk_d = nc.dram_tensor("wqkT", [DIM, 768], dt, kind="ExternalInput").ap()
    wv_d = nc.dram_tensor("wvT", [DIM, DIM], dt, kind="ExternalInput").ap()
    wp_d = nc.dram_tensor("wpT", [DIM, DIM], dt, kind="ExternalInput").ap()
    eb_d = nc.dram_tensor("ebT", [128, 1536], dt, kind="ExternalInput").ap()
    pb_d = nc.dram_tensor("pb", [PAIR, DIM], f32, kind="ExternalInput").ap()
    id_d = nc.dram_tensor("ident", [PAIR, PAIR], dt, kind="ExternalInput").ap()
    out_d = nc.dram_tensor("out", [tok_per_core, DIM], f32, kind="ExternalOutput").ap()

    n_super = tok_per_core // SUPER

    with tile.TileContext(nc) as tc:
        with (
            tc.tile_pool(name="consts", bufs=1) as consts,
            tc.tile_pool(name="xt", bufs=4) as xt_pool,
            tc.tile_pool(name="qk", bufs=3) as qk_pool,
            tc.tile_pool(name="er", bufs=3) as er_pool,
            tc.tile_pool(name="et", bufs=3) as et_pool,
            tc.tile_pool(name="av", bufs=4) as av_pool,
            tc.tile_pool(name="avt", bufs=6) as avt_pool,
            tc.tile_pool(name="osb", bufs=3) as o_pool,
            tc.tile_pool(name="inv", bufs=4) as inv_pool,
            tc.tile_pool(name="psA", bufs=3, space="PSUM") as psA,
            tc.tile_pool(name="psS", bufs=3, space="PSUM") as psS,
            tc.tile_pool(name="psV", bufs=2, space="PSUM") as psV,
        ):
            wqk = consts.tile([128, 3, 768], dt)
            nc.sync.dma_start(wqk[:], wqk_d.rearrange("(a p) k -> p a k", p=128))
            wv = consts.tile([128, 3, DIM], dt)
            nc.sync.dma_start(wv[:], wv_d.rearrange("(a p) k -> p a k", p=128))
            wp = consts.tile([128, 3, DIM], dt)
            nc.sync.dma_start(wp[:], wp_d.rearrange("(a p) k -> p a k", p=128))
            eb2 = consts.tile([128, 3, 512], dt)
            nc.sync.dma_start(eb2[:], eb_d.rearrange("p (a k) -> p a k", a=3))
            vbufs = []
            for _i in range(6):
                vper = consts.tile([128, H, 33], dt, tag=f"vper{_i}")
                nc.vector.memset(vper[:, :, 32:33], 1.0)
                vbufs.append(vper)

            pending = []

            def flush_pair(item):
                avt, blk, osb_t, pt0 = item
                ops = psA.tile([128, DIM], f32, tag="psA")
                for cc in range(3):
                    nc.tensor.matmul(
                        ops[:],
                        lhsT=avt[:, cc, :],
                        rhs=wp[:, cc, :],
                        start=(cc == 0),
                        stop=(cc == 2),
                    )
                nc.vector.tensor_copy(osb_t[:, blk, :], ops[:])
                if blk == 3:
                    out_ap = out_d[pt0 : pt0 + SUPER, :].rearrange(
                        "(b p) c -> p b c", p=128
                    )
                    if store_cast:
                        nc.gpsimd.dma_start(out_ap, osb_t[:])
                    else:
                        nc.sync.dma_start(out_ap, osb_t[:])

            for sp in range(n_super * reps):
                t0 = (sp % n_super) * SUPER
                xt = xt_pool.tile([128, 3, SUPER], dt, tag="xt")
                for cc in range(3):
                    nc.sync.dma_start(
                        out=xt[:, cc, :],
                        in_=x_d[t0 : t0 + SUPER, cc * 128 : (cc + 1) * 128],
                        transpose=True,
                    )

                qkt = qk_pool.tile([128, 6, SUPER], dt, tag="qkt")
                for j in range(6):
                    ps = psA.tile([128, SUPER], f32, tag="psA")
                    for cc in range(3):
                        nc.tensor.matmul(
                            ps[:],
                            lhsT=wqk[:, cc, j * 128 : (j + 1) * 128],
                            rhs=xt[:, cc, :],
                            start=(cc == 0),
                            stop=(cc == 2),
                        )
                    nc.scalar.copy(qkt[:, j, :], ps[:])

                for blk in range(4):
                    vps = psA.tile([128, DIM], f32, tag="psA")
                    for cc in range(3):
                        nc.tensor.matmul(
                            vps[:],
                            lhsT=xt[:, cc, blk * 128 : (blk + 1) * 128],
                            rhs=wv[:, cc, :],
                            start=(cc == 0),
                            stop=(cc == 2),
                        )
                    vsb = vbufs[(sp * 4 + blk) % 6]
                    nc.vector.tensor_copy(
                        vsb[:, :, 0:32], vps[:].rearrange("p (h d) -> p h d", d=32)
                    )
                osb_sup = o_pool.tile([128, 4, DIM], dt if store_cast else f32,
                                      tag="osb")

                for half in range(2):
                    er2 = er_pool.tile([128, 3, 512], dt, tag="er")
                    et2 = et_pool.tile([128, 3, 512], dt, tag="et")
                    for j in range(3):
                        st_j = psS.tile([128, 512], f32, tag="s")
                        for pr in range(2):
                            blk = 2 * half + pr
                            for g in range(4):
                                rp = g * 32
                                for w in range(2):
                                    f0 = blk * PAIR + w * N
                                    c0 = pr * 256 + g * 64
                                    nc.tensor.matmul(
                                        st_j[w * N : (w + 1) * N, c0 : c0 + 64],
                                        lhsT=qkt[rp : rp + 32, 3 + j, f0 : f0 + N],
                                        rhs=qkt[rp : rp + 32, j, f0 : f0 + N],
                                        start=True,
                                        stop=True,
                                        tile_position=(rp, w * N),
                                    )
                        nc.scalar.activation(
                            er2[:, j, :], st_j[:], mybir.ActivationFunctionType.Exp
                        )
                        nc.vector.tensor_mul(et2[:, j, :], er2[:, j, :], eb2[:, j, :])

                    for pr in range(2):
                        blk = 2 * half + pr
                        vsb = vbufs[(sp * 4 + blk) % 6]
                        avp_t = psV.tile([128, H * 33], f32, tag="avp")
                        avp = avp_t[:].rearrange("p (h d) -> p h d", d=33)
                        for h in range(H):
                            g, j = h % 4, h // 4
                            c0 = pr * 256 + g * 64
                            for w in range(2):
                                nc.tensor.matmul(
                                    avp[w * N : (w + 1) * N, h, :],
                                    lhsT=et2[w * N : (w + 1) * N, j, c0 : c0 + 64],
                                    rhs=vsb[w * N : (w + 1) * N, h, 0:33],
                                    start=True,
                                    stop=True,
                                    tile_position=(w * N, w * N),
                                )
                        inv = inv_pool.tile([128, H], f32, tag="inv")
                        nc.vector.reciprocal(inv[:], avp[:, :, 32])
                        avsb = av_pool.tile([128, H, 32], dt, tag="av")
                        nc.vector.tensor_mul(
                            avsb[:],
                            avp[:, :, 0:32],
                            inv[:, :, None].broadcast_to([128, H, 32]),
                        )
                        avt = avt_pool.tile([128, 3, 128], dt, tag="avt")
                        nc.scalar.dma_start(
                            out=avt[:],
                            in_=avsb[:].rearrange("p h d -> p (h d)"),
                            transpose=True,
                        )
                        pending.append((avt, blk, osb_sup, t0))
                        while len(pending) > 2:
                            flush_pair(pending.pop(0))
            while pending:
                flush_pair(pending.pop(0))
    nc.compile()
    return nc


def _get_nc(mode, tok_per_core, reps=1):
    key = (mode, tok_per_core, reps)
    if key not in _cache:
        if mode in ("v7", "v7s"):
            _cache[key] = _build_v7(tok_per_core, reps, store_cast=(mode == "v7"))
        elif mode.startswith(("v2", "v3", "v4", "v5", "v6", "v8")):
            _cache[key] = _build_v2(
                tok_per_core,
                reps,
                avt_pe=mode.endswith("pe"),
                dma_split=not mode.startswith("v2"),
                v4=mode.startswith(("v4", "v5", "v6", "v8")),
                bank_major=mode.startswith(("v5", "v6")),
                pool_mul=(mode == "v5p"),
                v6=mode.startswith(("v6", "v8")),
                pipe_tail=mode.startswith("v8"),
                cast_store=False if mode == "v8f" else None,
            )
        else:
            _cache[key] = _build(mode, tok_per_core, reps)
    return _cache[key]


def _host_prep(x, qkv_w, proj_w, proj_b, bias_table, rel_idx, mode, n_cores):
    np_dt = np.float32 if mode == "f32" else ml_dtypes.bfloat16
    x = np.asarray(x, np.float32)
    qkv_w = np.asarray(qkv_w, np.float32)
    proj_w = np.asarray(proj_w, np.float32)
    proj_b = np.asarray(proj_b, np.float32)
    bias_table = np.asarray(bias_table, np.float32)
    rel_idx = np.asarray(rel_idx)

    wq = qkv_w[0:DIM] * SCALE
    wk = qkv_w[DIM : 2 * DIM]
    wv = qkv_w[2 * DIM :]
    wqkT = np.concatenate([wq, wk], 0).T.copy().astype(np_dt)  # [384, 768]
    wvT = wv.T.copy().astype(np_dt)
    wpT = proj_w.T.copy().astype(np_dt)

    bias = bias_table[rel_idx.reshape(-1)].reshape(N, N, H)  # [nq, nk, h]
    eb1 = np.exp(bias).transpose(1, 2, 0)  # [nk, h, nq]
    if mode.startswith("v7"):
        # eb2 [128, (j, pr, g, q)]: rows repeat at 64 (same for both windows)
        ebT = np.zeros((PAIR, 3, 2, 4, N), np.float32)
        for h in range(H):
            g, j = h % 4, h // 4
            for w in range(2):
                for pr in range(2):
                    ebT[w * N : (w + 1) * N, j, pr, g, :] = eb1[:, h, :]
        ebT = ebT.reshape(PAIR, H * PAIR).astype(np_dt)
        ident = np.eye(PAIR, dtype=np_dt)
    elif mode.startswith(("v2", "v3", "v4", "v5", "v6", "v8")):
        # eb2 [128, (g, pr, j, q)]: rows repeat at 64 (same for both windows)
        ebT = np.zeros((PAIR, 4, 2, 3, N), np.float32)
        for h in range(H):
            g, j = h % 4, h // 4
            for w in range(2):
                for pr in range(2):
                    ebT[w * N : (w + 1) * N, g, pr, j, :] = eb1[:, h, :]
        ebT = ebT.reshape(PAIR, H * PAIR).astype(np_dt)
        ident = np.eye(PAIR, dtype=np_dt)
    else:
        ebT = np.zeros((PAIR, H * PAIR), np.float32)  # cross-window blocks stay 0
        for h in range(H):
            ec = (h % 4) * 384 + (h // 4) * 128
            for w in range(2):
                ebT[w * N : (w + 1) * N, ec + w * N : ec + (w + 1) * N] = eb1[:, h, :]
        ebT = ebT.astype(np_dt)  # [128, 1536]
        ident = np.eye(PAIR, dtype=np.float32)
    pb = np.broadcast_to(proj_b, (PAIR, DIM)).copy().astype(np.float32)

    B = x.shape[0]
    bper = B // n_cores
    xs = x.reshape(B * N, DIM).astype(np_dt)
    in_maps = []
    for c in range(n_cores):
        in_maps.append(
            {
                "x": xs[c * bper * N : (c + 1) * bper * N],
                "wqkT": wqkT,
                "wvT": wvT,
                "wpT": wpT,
                "ebT": ebT,
                "pb": pb,
                "ident": ident,
            }
        )
    return in_maps


def kernel(x, qkv_w, proj_w, proj_b, bias_table, rel_idx):
    from concourse.bass_utils import run_bass_kernel_spmd

    x = np.asarray(x)
    B = x.shape[0]
    n_cores = NCORES
    tok_per_core = (B // n_cores) * N
    nc = _get_nc(MODE, tok_per_core)
    in_maps = _host_prep(x, qkv_w, proj_w, proj_b, bias_table, rel_idx, MODE, n_cores)
    res = run_bass_kernel_spmd(nc, in_maps, list(range(n_cores)))
    out = np.concatenate([r["out"] for r in res.results], 0)
    return out.reshape(B, N, DIM).astype(np.float32)

